# revision 1
# baseline (speedup 1.0000x reference)
"""TRN2 Bass/Tile kernel for nn_NoFoDifformer (8-core SPMD, row-sharded).

v10 design (lineage: v1 ~374us / v6 ~127 / v8 ~122 / v9 ~114 / v10 ~75us):
  - u is shipped from the host in BOTH layouts as fp8e4m3 (IEEE variant,
    max 240!), host-scaled x16 to sit in fp8 range: u8_s [ROWS,N] (row
    slice, pass 1) and uT8_s [N,ROWS] (pre-transposed, pass 2). This kills
    the DRAM scratch round-trip, DMA-transpose reads, the SBUF stash + 512
    PE transposes and SWDGE cast-loads of v1, and halves u HBM traffic to
    16MB/core of plain HWDGE streaming. The output is attention-dominated
    (h_fur absmax 0.12 vs output 3.3e4) so u-path precision is free.
  - pass1/pass2 matmuls run fp8 DoubleRow (2 contraction blocks per
    instruction, exact arithmetic, PE time halved to ~49us modeled).
  - utx is AllReduced in fp8 (utx16 absmax 128 < 240): NAR chunks of
    0.25MB ride the panel stream; g = new_e*utx carries the second x16;
    1/256 is folded into the h_fur psum copy. utxT tiles stage through a
    bf16 copy before PE transpose (the verifier rejects fp8 transpose
    outputs to PSUM).
  - Collective SPAD order matters (ncfw executes in program order, each
    with a ~10us floor): AR0, AR1, kTv last (consumed post-pass2). NAR=2
    beats 4 (fewer floors). The new_e AllGather is ELIMINATED: AllGather ==
    AllReduce of zero-padded buffers, so 16*ne (fp8) rides AR0 as JT extra
    columns, placed via a host-provided per-core mask (16.0 on own jt
    block) times a free-dim broadcast of the sharded sine result.
    The LN1/q/k/v/kTv head is emitted INSIDE the panel loop (after chunk
    0's panels) so chunk 0's matmuls -> AR0 input aren't delayed by head
    PE work. gpsimd queue carries ONLY collective triggers.
  - x is shipped pre-transposed as bf16 (xT16_s); encoder + q/k/v/kTv
    matmuls are bf16-operand (f32 PSUM accum); attT/sT stay f32 (they ARE
    the output scale); FFN matmuls bf16. Measured rel err: 7.2e-3 (gate
    2e-2), HW-validated.
  - Queue plan (deadlock-free by construction): sync HWDGE = 8x 1MB
    panels, AR in-copies, then ALL collective load-backs after the last
    panel, then y. scalar HWDGE = xT16 + encoder weights FIRST (the head
    gates pass1 gates AR0), other weights, then the 8x 1MB uT prefetch
    stream (bufs=2, NOT more: deeper prefetch contends with the panel
    stream on the shared SDMA engines, delaying AR0's input and shifting
    the whole serialized collective chain — measured 39.6us/body in a
    same-session A/B, the 2nd-largest single win. pass2 only needs uT
    from ~44us and its consumption outpaces the DMA, so 2 bufs never
    starve it). PE: pass1 matmuls -> 64 utxT transposes/pass2 matmuls ->
    attention tail.
"""

import os as _os

import numpy as np

import concourse.bacc as bacc
import concourse.mybir as mybir
import concourse.tile as tile
from concourse.bass_utils import run_bass_kernel_spmd
from concourse.masks import make_identity

F32 = mybir.dt.float32
F32R = mybir.dt.float32r
F8 = mybir.dt.float8e4
BF16 = mybir.dt.bfloat16
AF = mybir.ActivationFunctionType
ALU = mybir.AluOpType

NCORES = 8
N = 8192
NFEAT = 512
HID = 256
C = 128
DIM = 32
KPOW = 10
ROWS = N // NCORES      # 1024 rows per core
NT = ROWS // 128        # 8 row tiles
JT = N // 128           # 64 column tiles
PW = 1024               # pass-1 panel width
JP = N // PW            # 16 panels
JTC = JT // NCORES      # 8 jt per core for new_e sharding
UTG = 8                 # jt tiles per uT prefetch DMA
LN_EPS = 1e-5

TWO_PI = 6.283185307179586
INV_2PI = 1.0 / TWO_PI
CW_C1 = 6.28125
CW_C2 = float(np.float32(TWO_PI - CW_C1))
CW_C3 = TWO_PI - CW_C1 - CW_C2
MAGIC = 12582912.0      # 1.5 * 2**23, round-to-nearest trick
HALF_PI = float(np.float32(np.pi / 2))
PI_F = float(np.float32(np.pi))

WEIGHT_NAMES = [
    ("fe_w1", [NFEAT, HID]), ("fe_b1", [HID]),
    ("fe_w2", [HID, C]), ("fe_b2", [C]),
    ("eig_w", [KPOW, DIM + 1]), ("eig_b", [KPOW]), ("alpha_w", [KPOW]),
    ("mha_g", [C]), ("mha_b", [C]), ("ffn_g", [C]), ("ffn_b", [C]),
    ("wq", [C, C]), ("bq", [C]), ("wk", [C, C]), ("bk", [C]),
    ("wv", [C, C]), ("bv", [C]), ("wo", [C, C]), ("bo", [C]),
    ("f1_w", [C, C]), ("f1_b", [C]), ("f2_w", [C, C]), ("f2_b", [C]),
]


def _build(nc):
    io = {}
    io["u8_s"] = nc.dram_tensor("u8_s", [ROWS, N], F8, kind="ExternalInput")
    io["uT8_s"] = nc.dram_tensor("uT8_s", [N, ROWS], F8, kind="ExternalInput")
    io["xT16_s"] = nc.dram_tensor("xT16_s", [NFEAT, ROWS], BF16, kind="ExternalInput")
    io["e_js"] = nc.dram_tensor("e_js", [128, JTC], F32, kind="ExternalInput")
    io["ne_mask"] = nc.dram_tensor("ne_mask", [128, JT], F32, kind="ExternalInput")
    for name, shape in WEIGHT_NAMES:
        io[name] = nc.dram_tensor(name, shape, F32, kind="ExternalInput")
    y = nc.dram_tensor("y", [ROWS, C], F32, kind="ExternalOutput")

    div_const = nc.inline_tensor(
        np.tile(np.arange(1, DIM // 2 + 1, dtype=np.float32), (128, 1)), name="divc"
    )

    NAR = int(_os.environ.get("KERNEL_NAR", "2"))
    _REPL = int(_os.environ.get("KERNEL_REPLICATE", "1"))
    JPC = JP // NAR              # panels per AR chunk
    CW = N // NAR                # utxT columns per AR chunk

    with tile.TileContext(nc) as tc:
        with (
            tc.tile_pool(name="persist", bufs=1) as per,
            tc.tile_pool(name="pan", bufs=4) as pan,
            tc.tile_pool(name="u16t", bufs=2) as u16t_pool,
            tc.tile_pool(name="g16p", bufs=4) as g16_pool,
            tc.tile_pool(name="t16p", bufs=3) as t16_pool,
            tc.tile_pool(name="stats", bufs=4) as stats,
            tc.tile_pool(name="psum_sm", bufs=3, space="PSUM") as psum_sm,
            tc.tile_pool(name="psum_tr", bufs=2, space="PSUM") as psum_tr,
            tc.tile_pool(name="psum_acc", bufs=1, space="PSUM") as psum_acc,
            tc.tile_pool(name="dram", bufs=1, space="DRAM") as dram,
        ):
            def _body_once():
                rg = [list(range(NCORES))]

                # ---------------- constants / weights ----------------
                ident = per.tile([128, 128], F32, tag="ident")
                make_identity(nc, ident[:])
                ident16 = per.tile([128, 128], BF16, tag="ident16")
                make_identity(nc, ident16[:])

                eps_sb = per.tile([128, 1], F32, tag="eps_sb")
                nc.vector.memset(eps_sb[:], LN_EPS)

                div_sb = per.tile([128, DIM // 2], F32, tag="div_sb")
                nc.scalar.dma_start(out=div_sb[:], in_=div_const.ap())

                def bcast(name, width, tag):
                    t = per.tile([128, width], F32, tag=tag)
                    src = io[name].ap()
                    if len(src.shape) > 1:
                        src = src.rearrange("a b -> (a b)")
                    nc.scalar.dma_start(out=t[:], in_=src.partition_broadcast(128))
                    return t

                def per_part(name, tag):
                    t = per.tile([128, 1], F32, tag=tag)
                    nc.scalar.dma_start(out=t[:], in_=io[name].ap().rearrange("(p o) -> p o", o=1))
                    return t

                def load16(name, shape2, tag, rearr=None):
                    """Load an f32 weight then cast to a bf16 SBUF tile."""
                    tf = per.tile(shape2, F32, tag=tag + "_f")
                    src = io[name].ap()
                    if rearr is not None:
                        src = src.rearrange(*rearr[:1], **rearr[1])
                    nc.scalar.dma_start(out=tf[:], in_=src)
                    tb = per.tile(shape2, BF16, tag=tag)
                    nc.vector.tensor_copy(out=tb[:], in_=tf[:])
                    return tb


                # encoder inputs FIRST on the scalar ring: the head gates
                # pass1 which gates AR0
                xT16 = per.tile([128, NFEAT // 128, ROWS], BF16, tag="xT16")
                nc.scalar.dma_start(out=xT16[:], in_=io["xT16_s"].ap().rearrange("(t p) r -> p t r", p=128))
                w1_16 = load16("fe_w1", [128, NFEAT // 128, HID], "w1_16",
                               ("(t p) h -> p t h", dict(p=128)))
                w2_16 = load16("fe_w2", [128, HID // 128, C], "w2_16",
                               ("(t p) c -> p t c", dict(p=128)))
                b1_sb = per.tile([128, HID // 128], F32, tag="b1_sb")
                nc.scalar.dma_start(out=b1_sb[:], in_=io["fe_b1"].ap().rearrange("(t p) -> p t", p=128))
                b2_bc = bcast("fe_b2", C, "b2_bc")

                wq_16 = load16("wq", [128, C], "wq_16")
                wk_16 = load16("wk", [128, C], "wk_16")
                wv_16 = load16("wv", [128, C], "wv_16")
                wo_sb = per.tile([128, C], F32, tag="wo_sb")
                nc.scalar.dma_start(out=wo_sb[:], in_=io["wo"].ap())
                f1w_16 = load16("f1_w", [128, C], "f1w_16")
                f2w_16 = load16("f2_w", [128, C], "f2w_16")

                bq_pp = per_part("bq", "bq_pp")
                bo_pp = per_part("bo", "bo_pp")
                f1b_pp = per_part("f1_b", "f1b_pp")
                f2b_pp = per_part("f2_b", "f2b_pp")
                bk_bc = bcast("bk", C, "bk_bc")
                bv_bc = bcast("bv", C, "bv_bc")
                mhag_bc = bcast("mha_g", C, "mhag_bc")
                mhab_bc = bcast("mha_b", C, "mhab_bc")
                ffng_bc = bcast("ffn_g", C, "ffng_bc")
                ffnb_bc = bcast("ffn_b", C, "ffnb_bc")


                # ---------------- feat encoder (bf16 matmuls) ----------------
                # t1^T [hid_part, 2(ht), n] = relu(w1^T x^T + b1)
                t1T16 = per.tile([128, HID // 128, ROWS], BF16, tag="t1T16")
                for ht in range(HID // 128):
                    for nch in range(ROWS // 512):
                        ps = psum_sm.tile([128, 512], F32, tag="ps_sm")
                        for ft in range(NFEAT // 128):
                            nc.tensor.matmul(
                                ps[:], lhsT=w1_16[:, ft, ht * 128:(ht + 1) * 128],
                                rhs=xT16[:, ft, nch * 512:(nch + 1) * 512],
                                start=(ft == 0), stop=(ft == NFEAT // 128 - 1),
                            )
                        nc.scalar.activation(
                            out=t1T16[:, ht, nch * 512:(nch + 1) * 512], in_=ps[:],
                            func=AF.Relu, bias=b1_sb[:, ht:ht + 1],
                        )

                # h [n_part, 8(nt), C] = t1 @ w2 + b2 (keep f32 + bf16 copies)
                h_sb = per.tile([128, NT, C], F32, tag="h_sb")
                h8_sb = per.tile([128, NT, C], F8, tag="h8_sb")
                for nt in range(NT):
                    ps = psum_sm.tile([128, C], F32, tag="ps_sm")
                    for ht in range(HID // 128):
                        nc.tensor.matmul(
                            ps[:], lhsT=t1T16[:, ht, nt * 128:(nt + 1) * 128],
                            rhs=w2_16[:, ht, :],
                            start=(ht == 0), stop=(ht == HID // 128 - 1),
                        )
                    nc.vector.tensor_add(out=h_sb[:, nt, :], in0=ps[:], in1=b2_bc[:])
                    nc.vector.tensor_copy(out=h8_sb[:, nt, :], in_=h_sb[:, nt, :])

                # ---------------- new_e (jt-sharded) + AllGather ----------------
                eigw_bc = bcast("eig_w", KPOW * (DIM + 1), "eigw_bc")
                eigb_bc = bcast("eig_b", KPOW, "eigb_bc")
                alpha_bc = bcast("alpha_w", KPOW, "alpha_bc")

                w2s = per.tile([128, KPOW, DIM // 2], F32, tag="w2s")
                w2c = per.tile([128, KPOW, DIM // 2], F32, tag="w2c")
                eigw_3d = eigw_bc[:].rearrange("p (k d) -> p k d", d=DIM + 1)
                alpha_b3 = alpha_bc[:].unsqueeze(2).broadcast_to([128, KPOW, DIM // 2])
                nc.vector.tensor_tensor(out=w2s[:], in0=alpha_b3, in1=eigw_3d[:, :, 1:1 + DIM // 2], op=ALU.mult)
                nc.vector.tensor_tensor(out=w2c[:], in0=alpha_b3, in1=eigw_3d[:, :, 1 + DIM // 2:DIM + 1], op=ALU.mult)
                w0t = per.tile([128, KPOW], F32, tag="w0t")
                nc.vector.tensor_tensor(out=w0t[:], in0=eigw_3d[:, :, 0], in1=eigb_bc[:], op=ALU.add)
                nc.vector.tensor_tensor(out=w0t[:], in0=w0t[:], in1=alpha_bc[:], op=ALU.mult)
                w0 = per.tile([128, 1], F32, tag="w0")
                nc.vector.tensor_reduce(out=w0[:], in_=w0t[:], axis=mybir.AxisListType.X, op=ALU.add)

                e_sb = per.tile([128, JTC], F32, tag="e_sb")
                nc.scalar.dma_start(out=e_sb[:], in_=io["e_js"].ap())
                pows = per.tile([128, JTC, KPOW], F32, tag="pows")
                nc.vector.tensor_copy(out=pows[:, :, 0], in_=e_sb[:])
                for k in range(1, KPOW):
                    nc.vector.tensor_tensor(out=pows[:, :, k], in0=pows[:, :, k - 1], in1=e_sb[:], op=ALU.mult)

                WNE = JTC * KPOW * (DIM // 2)  # 1280
                pe_t = per.tile([128, JTC, KPOW, DIM // 2], F32, tag="pe_t")
                kq_t = per.tile([128, WNE], F32, tag="kq_t")
                trig = per.tile([128, WNE], F32, tag="trig")
                ne_s = per.tile([128, JTC], F32, tag="ne_s")
                ne_c = per.tile([128, JTC], F32, tag="ne_c")

                pows_b = pows[:].unsqueeze(3).broadcast_to([128, JTC, KPOW, DIM // 2])
                div_b = div_sb[:].unsqueeze(1).unsqueeze(1).broadcast_to([128, JTC, KPOW, DIM // 2])
                nc.vector.tensor_tensor(out=pe_t[:], in0=pows_b, in1=div_b, op=ALU.mult)
                pe_f = pe_t[:].rearrange("p a b c -> p (a b c)")
                nc.vector.tensor_scalar(out=kq_t[:], in0=pe_f, scalar1=INV_2PI, scalar2=MAGIC, op0=ALU.mult, op1=ALU.add)
                nc.vector.tensor_scalar_sub(out=kq_t[:], in0=kq_t[:], scalar1=MAGIC)
                nc.vector.cody_waite_cascade(pe_f, pe_f, kq_t[:], CW_C1, CW_C2, CW_C3)

                w2s_b = w2s[:].rearrange("p k d -> p (k d)").unsqueeze(1).broadcast_to([128, JTC, KPOW * DIM // 2])
                w2c_b = w2c[:].rearrange("p k d -> p (k d)").unsqueeze(1).broadcast_to([128, JTC, KPOW * DIM // 2])

                nc.scalar.activation(out=trig[:], in_=pe_f, func=AF.Sin)
                trig3 = trig[:].rearrange("p (a w) -> p a w", a=JTC)
                nc.vector.tensor_tensor(out=trig3, in0=trig3, in1=w2s_b, op=ALU.mult)
                nc.vector.tensor_reduce(out=ne_s[:], in_=trig3, axis=mybir.AxisListType.X, op=ALU.add)

                nc.vector.add_range_wrap(kq_t[:], pe_f, HALF_PI, PI_F, TWO_PI)
                nc.scalar.activation(out=trig[:], in_=kq_t[:], func=AF.Sin)
                nc.vector.tensor_tensor(out=trig3, in0=trig3, in1=w2c_b, op=ALU.mult)
                nc.vector.tensor_reduce(out=ne_c[:], in_=trig3, axis=mybir.AxisListType.X, op=ALU.add)

                nc.vector.tensor_tensor(out=ne_s[:], in0=ne_s[:], in1=ne_c[:], op=ALU.add)
                nc.vector.tensor_scalar_add(out=ne_s[:], in0=ne_s[:], scalar1=w0[:])

                # ne placed into fp8 via host mask (16.0 on own jt block):
                # AllGather == AllReduce of zero-padded buffers
                mask_sb = per.tile([128, JT], F32, tag="mask_sb")
                nc.scalar.dma_start(out=mask_sb[:], in_=io["ne_mask"].ap())
                ne8_placed = per.tile([128, JT], F8, tag="ne8_placed")
                ne_bc3 = ne_s[:].unsqueeze(1).broadcast_to([128, NCORES, JTC])
                nc.vector.tensor_tensor(
                    out=ne8_placed[:].rearrange("p (r b) -> p r b", b=JTC),
                    in0=ne_bc3,
                    in1=mask_sb[:].rearrange("p (r b) -> p r b", b=JTC),
                    op=ALU.mult)
                ne8_lb = per.tile([128, JT], F8, tag="ne8_lb")
                new_e_sb = per.tile([128, JT], F32, tag="new_e_sb")

                # ---------------- LN1 + q/k/v + kTv partial ----------------
                def layer_norm(src, dst, g_bc, b_bc):
                    for nt in range(NT):
                        st = stats.tile([128, 6], F32, tag="ln_st")
                        nc.vector.bn_stats(out=st[:], in_=src[:, nt, :])
                        mv = stats.tile([128, 2], F32, tag="ln_mv")
                        nc.vector.bn_aggr(out=mv[:], in_=st[:])
                        rstd = stats.tile([128, 1], F32, tag="ln_rstd")
                        nc.scalar.activation(out=rstd[:], in_=mv[:, 1:2], func=AF.Sqrt, bias=eps_sb[:])
                        nc.vector.reciprocal(out=rstd[:], in_=rstd[:])
                        nc.vector.tensor_scalar(
                            out=dst[:, nt, :], in0=src[:, nt, :],
                            scalar1=mv[:, 0:1], scalar2=rstd[:],
                            op0=ALU.subtract, op1=ALU.mult,
                        )
                        nc.vector.tensor_tensor(out=dst[:, nt, :], in0=dst[:, nt, :], in1=g_bc[:], op=ALU.mult)
                        nc.vector.tensor_tensor(out=dst[:, nt, :], in0=dst[:, nt, :], in1=b_bc[:], op=ALU.add)

                mh_sb = per.tile([128, NT, C], F32, tag="mh_sb")

                mh16T = per.tile([128, ROWS], BF16, tag="mh16T")
                qT = per.tile([128, ROWS], F32, tag="qT")
                k16_sb = per.tile([128, NT, C], BF16, tag="k16_sb")
                v16_sb = per.tile([128, NT, C], BF16, tag="v16_sb")
                kTv_sb = per.tile([128, C], F32, tag="kTv_sb")

                def emit_qkv_head():
                    layer_norm(h_sb, mh_sb, mhag_bc, mhab_bc)
                    for nt in range(NT):
                        tp = psum_tr.tile([128, 128], F32, tag="tr")
                        nc.tensor.transpose(tp[:], mh_sb[:, nt, :], ident[:])
                        nc.vector.tensor_copy(out=mh16T[:, nt * 128:(nt + 1) * 128], in_=tp[:])
                    for nch in range(ROWS // 512):
                        ps = psum_sm.tile([128, 512], F32, tag="ps_sm")
                        nc.tensor.matmul(ps[:], lhsT=wq_16[:], rhs=mh16T[:, nch * 512:(nch + 1) * 512], start=True, stop=True)
                        nc.scalar.activation(out=qT[:, nch * 512:(nch + 1) * 512], in_=ps[:], func=AF.Identity, bias=bq_pp[:])
                    for nt in range(NT):
                        ps = psum_sm.tile([128, C], F32, tag="ps_sm")
                        nc.tensor.matmul(ps[:], lhsT=mh16T[:, nt * 128:(nt + 1) * 128], rhs=wk_16[:], start=True, stop=True)
                        nc.vector.tensor_add(out=k16_sb[:, nt, :], in0=ps[:], in1=bk_bc[:])
                        ps2 = psum_sm.tile([128, C], F32, tag="ps_sm")
                        nc.tensor.matmul(ps2[:], lhsT=mh16T[:, nt * 128:(nt + 1) * 128], rhs=wv_16[:], start=True, stop=True)
                        nc.vector.tensor_add(out=v16_sb[:, nt, :], in0=ps2[:], in1=bv_bc[:])
                    pskv = psum_sm.tile([128, C], F32, tag="ps_sm")
                    for nt in range(NT):
                        nc.tensor.matmul(pskv[:], lhsT=k16_sb[:, nt, :], rhs=v16_sb[:, nt, :], start=(nt == 0), stop=(nt == NT - 1))
                    nc.vector.tensor_copy(out=kTv_sb[:], in_=pskv[:])

                # ---------------- collectives (triggers on gpsimd only) ----------------
                utxT = per.tile([128, N], F8, tag="utxT")
                ar_ins, ar_outs = [], []
                for c in range(NAR):
                    w = CW + (JT if c == 0 else 0)  # chunk 0 carries ne columns
                    ari = dram.tile([128, w], F8, tag=f"ar_in{c}", name=f"ar_in{c}")
                    aro = dram.tile([128, w], F8, tag=f"ar_out{c}", name=f"ar_out{c}",
                                    addr_space="Shared")
                    ar_ins.append(ari)
                    ar_outs.append(aro)
                ktv_in = dram.tile([128, C], F32, tag="ktv_in")
                ktv_out = dram.tile([128, C], F32, tag="ktv_out", addr_space="Shared")

                def emit_ktv_trigger():
                    nc.sync.dma_start(out=ktv_in[:], in_=kTv_sb[:])
                    nc.gpsimd.collective_compute(
                        "AllReduce", ALU.add, replica_groups=rg,
                        ins=[ktv_in[:].opt()], outs=[ktv_out[:].opt()],
                    )

                def emit_ar_trigger(c):
                    nc.sync.dma_start(out=ar_ins[c][:, 0:CW], in_=utxT[:, c * CW:(c + 1) * CW])
                    if c == 0:
                        nc.sync.dma_start(out=ar_ins[0][:, CW:CW + JT], in_=ne8_placed[:])
                    nc.gpsimd.collective_compute(
                        "AllReduce", ALU.add, replica_groups=rg,
                        ins=[ar_ins[c][:].opt()], outs=[ar_outs[c][:].opt()],
                    )

                # ---------------- pass 1: utx^T = h16^T @ u panels ----------------
                u_r = io["u8_s"].ap().rearrange("(t p) j -> p t j", p=128)
                for jp in range(JP):
                    panel = pan.tile([128, NT, PW], F8, tag="panel")
                    nc.sync.dma_start(out=panel[:], in_=u_r[:, :, jp * PW:(jp + 1) * PW])
                    if jp == min(JPC, JP - 1):
                        emit_qkv_head()
                    if jp >= JPC + 1 and (jp - JPC - 1) % JPC == 0 and (jp - JPC - 1) // JPC < NAR - 1:
                        emit_ar_trigger((jp - JPC - 1) // JPC)
                    for jh in range(PW // 512):
                        ps = psum_sm.tile([128, 512], F32, tag="ps_sm")
                        for ntp in range(NT // 2):
                            nc.tensor.matmul(
                                ps[:], lhsT=h8_sb[:, 2 * ntp:2 * ntp + 2, :],
                                rhs=panel[:, 2 * ntp:2 * ntp + 2, jh * 512:(jh + 1) * 512],
                                start=(ntp == 0), stop=(ntp == NT // 2 - 1),
                                perf_mode=mybir.MatmulPerfMode.DoubleRow,
                            )
                        nc.scalar.activation(
                            out=utxT[:, jp * PW + jh * 512:jp * PW + (jh + 1) * 512],
                            in_=ps[:], func=AF.Identity)
                emit_ar_trigger(NAR - 1)
                emit_ktv_trigger()

                # all collective load-backs, in completion order, after the last
                # panel so they never block the panel stream's FIFO
                nc.sync.dma_start(out=utxT[:, 0:CW], in_=ar_outs[0][:, 0:CW])
                nc.sync.dma_start(out=ne8_lb[:], in_=ar_outs[0][:, CW:CW + JT])
                nc.vector.tensor_copy(out=new_e_sb[:], in_=ne8_lb[:])
                for c in range(1, NAR):
                    nc.sync.dma_start(out=utxT[:, c * CW:(c + 1) * CW], in_=ar_outs[c][:])
                nc.sync.dma_start(out=kTv_sb[:], in_=ktv_out[:])

                # ---------------- pass 2: h_fur^T = sum_jt g16[jt]^T @ uT16[jt] ----------------
                uT_r = io["uT8_s"].ap().rearrange("(jt p) r -> p jt r", p=128)
                hfur_ps = psum_acc.tile([128, ROWS], F32, tag="hfur")
                for jtg in range(JT // UTG):
                    ut = u16t_pool.tile([128, UTG, ROWS], F8, tag="ut")
                    nc.scalar.dma_start(out=ut[:], in_=uT_r[:, jtg * UTG:(jtg + 1) * UTG, :])
                    for jpr in range(UTG // 2):
                        g8p = g16_pool.tile([128, 2, 128], F8, tag="g8p")
                        for k in range(2):
                            jt = jtg * UTG + jpr * 2 + k
                            t16 = t16_pool.tile([128, 128], BF16, tag="t16")
                            nc.vector.tensor_copy(out=t16[:], in_=utxT[:, jt * 128:(jt + 1) * 128])
                            tp = psum_tr.tile([128, 128], BF16, tag="tr", name="tp16")
                            nc.tensor.transpose(tp[:], t16[:], ident16[:])
                            nc.vector.tensor_scalar_mul(out=g8p[:, k, :], in0=tp[:], scalar1=new_e_sb[:, jt:jt + 1])
                        pair = jtg * (UTG // 2) + jpr
                        for hf in range(ROWS // 512):
                            nc.tensor.matmul(
                                hfur_ps[:, hf * 512:(hf + 1) * 512], lhsT=g8p[:],
                                rhs=ut[:, jpr * 2:jpr * 2 + 2, hf * 512:(hf + 1) * 512],
                                start=(pair == 0), stop=(pair == JT // 2 - 1),
                                skip_group_check=True,
                                perf_mode=mybir.MatmulPerfMode.DoubleRow,
                            )

                # ---------------- att^T (fp32r), s^T, h1 ----------------
                hfurT = per.tile([128, ROWS], F32, tag="hfurT")
                nc.vector.tensor_scalar_mul(out=hfurT[:], in0=hfur_ps[:], scalar1=1.0 / 4096.0)

                attT = per.tile([128, ROWS], F32, tag="attT")
                for nch in range(ROWS // 512):
                    ps = psum_sm.tile([128, 512], F32, tag="ps_sm")
                    nc.tensor.matmul(ps[:], lhsT=kTv_sb[:],
                                     rhs=qT[:, nch * 512:(nch + 1) * 512],
                                     start=True, stop=True)
                    nc.vector.tensor_copy(out=attT[:, nch * 512:(nch + 1) * 512], in_=ps[:])

                sT = per.tile([128, ROWS], F32, tag="sT")
                for nch in range(ROWS // 512):
                    ps = psum_sm.tile([128, 512], F32, tag="ps_sm")
                    nc.tensor.matmul(ps[:], lhsT=wo_sb[:],
                                     rhs=attT[:, nch * 512:(nch + 1) * 512],
                                     start=True, stop=True)
                    nc.vector.scalar_tensor_tensor(
                        out=sT[:, nch * 512:(nch + 1) * 512], in0=ps[:], scalar=bo_pp[:],
                        in1=hfurT[:, nch * 512:(nch + 1) * 512],
                        op0=ALU.add, op1=ALU.add,
                    )

                h1_sb = per.tile([128, NT, C], F32, tag="h1_sb")
                for nt in range(NT):
                    tp = psum_tr.tile([128, 128], F32, tag="tr")
                    nc.tensor.transpose(tp[:], sT[:, nt * 128:(nt + 1) * 128], ident[:])
                    nc.vector.tensor_add(out=h1_sb[:, nt, :], in0=tp[:], in1=h_sb[:, nt, :])

                # ---------------- FFN ----------------
                mh2_sb = per.tile([128, NT, C], F32, tag="mh2_sb")
                layer_norm(h1_sb, mh2_sb, ffng_bc, ffnb_bc)
                mh2T = per.tile([128, ROWS], BF16, tag="mh2T")
                for nt in range(NT):
                    tp = psum_tr.tile([128, 128], F32, tag="tr")
                    nc.tensor.transpose(tp[:], mh2_sb[:, nt, :], ident[:])
                    nc.vector.tensor_copy(out=mh2T[:, nt * 128:(nt + 1) * 128], in_=tp[:])

                gzT = per.tile([128, ROWS], BF16, tag="gzT")
                for nch in range(ROWS // 512):
                    ps = psum_sm.tile([128, 512], F32, tag="ps_sm")
                    nc.tensor.matmul(ps[:], lhsT=f1w_16[:],
                                     rhs=mh2T[:, nch * 512:(nch + 1) * 512],
                                     start=True, stop=True)
                    nc.scalar.activation(out=gzT[:, nch * 512:(nch + 1) * 512], in_=ps[:], func=AF.Gelu, bias=f1b_pp[:])

                f2T = per.tile([128, ROWS], F32, tag="f2T")
                for nch in range(ROWS // 512):
                    ps = psum_sm.tile([128, 512], F32, tag="ps_sm")
                    nc.tensor.matmul(ps[:], lhsT=f2w_16[:],
                                     rhs=gzT[:, nch * 512:(nch + 1) * 512],
                                     start=True, stop=True)
                    nc.scalar.activation(out=f2T[:, nch * 512:(nch + 1) * 512], in_=ps[:], func=AF.Identity, bias=f2b_pp[:])

                hout_sb = per.tile([128, NT, C], F32, tag="hout_sb")
                for nt in range(NT):
                    tp = psum_tr.tile([128, 128], F32, tag="tr")
                    nc.tensor.transpose(tp[:], f2T[:, nt * 128:(nt + 1) * 128], ident[:])
                    nc.vector.tensor_add(out=hout_sb[:, nt, :], in0=tp[:], in1=h1_sb[:, nt, :])

                nc.sync.dma_start(out=y.ap().rearrange("(t p) c -> p t c", p=128), in_=hout_sb[:])

            for _rep in range(_REPL):
                _body_once()

    nc.compile()
    return nc


_NC = None


def _get_nc():
    global _NC
    if _NC is None:
        _NC = _build(bacc.Bacc("TRN2", target_bir_lowering=False, debug=False, num_devices=NCORES))
    return _NC


def make_in_maps(inputs):
    import ml_dtypes
    BF = ml_dtypes.bfloat16
    F8E4 = ml_dtypes.float8_e4m3

    e = np.ascontiguousarray(np.asarray(inputs["e"], dtype=np.float32))
    u = np.asarray(inputs["u"], dtype=np.float32)
    x = np.asarray(inputs["x"], dtype=np.float32)
    e_resh = np.ascontiguousarray(e.reshape(JT, 128).T)  # [p, jt] = e[jt*128+p]

    weights = {
        name: np.ascontiguousarray(np.asarray(inputs[name], dtype=np.float32))
        for name, _ in WEIGHT_NAMES
    }

    in_maps = []
    for m in range(NCORES):
        us = u[m * ROWS:(m + 1) * ROWS]
        xs = x[m * ROWS:(m + 1) * ROWS]
        us64 = us * np.float32(16.0)
        mask = np.zeros((128, JT), np.float32)
        mask[:, m * JTC:(m + 1) * JTC] = 16.0
        im = {
            "u8_s": us64.astype(F8E4),
            "uT8_s": us64.T.astype(F8E4),
            "ne_mask": mask,
            "xT16_s": xs.T.astype(BF),
            "e_js": np.ascontiguousarray(e_resh[:, m * JTC:(m + 1) * JTC]),
        }
        im.update(weights)
        in_maps.append(im)
    return in_maps


def kernel(**inputs):
    nc = _get_nc()
    in_maps = make_in_maps(inputs)

    trace = bool(int(_os.environ.get("KERNEL_TRACE", "0")))
    res = run_bass_kernel_spmd(nc, in_maps, core_ids=list(range(NCORES)), trace=trace)
    if trace and res.exec_time_ns is not None:
        print(f"HW exec time: {res.exec_time_ns} ns")
        if res.instructions_and_trace is not None:
            print("trace:", res.instructions_and_trace[1])
    out = np.concatenate([r["y"] for r in res.results], axis=0)
    return out.astype(np.float32)



# revision 8
# speedup vs baseline: 11.2630x; 11.2630x over previous
"""TRN2 Bass/Tile kernel for nn_NoFoDifformer (8-core SPMD, row-sharded).

v10 design (lineage: v1 ~374us / v6 ~127 / v8 ~122 / v9 ~114 / v10 ~75us):
  - u is shipped from the host in BOTH layouts as fp8e4m3 (IEEE variant,
    max 240!), host-scaled x16 to sit in fp8 range: u8_s [ROWS,N] (row
    slice, pass 1) and uT8_s [N,ROWS] (pre-transposed, pass 2). This kills
    the DRAM scratch round-trip, DMA-transpose reads, the SBUF stash + 512
    PE transposes and SWDGE cast-loads of v1, and halves u HBM traffic to
    16MB/core of plain HWDGE streaming. The output is attention-dominated
    (h_fur absmax 0.12 vs output 3.3e4) so u-path precision is free.
  - pass1/pass2 matmuls run fp8 DoubleRow (2 contraction blocks per
    instruction, exact arithmetic, PE time halved to ~49us modeled).
  - utx is AllReduced in fp8 (utx16 absmax 128 < 240): NAR chunks of
    0.25MB ride the panel stream; g = new_e*utx carries the second x16;
    1/256 is folded into the h_fur psum copy. utxT tiles stage through a
    bf16 copy before PE transpose (the verifier rejects fp8 transpose
    outputs to PSUM).
  - Collective SPAD order matters (ncfw executes in program order, each
    with a ~10us floor): AR0, AR1, kTv last (consumed post-pass2). NAR=2
    beats 4 (fewer floors). The new_e AllGather is ELIMINATED: AllGather ==
    AllReduce of zero-padded buffers, so 16*ne (fp8) rides AR0 as JT extra
    columns, placed via a host-provided per-core mask (16.0 on own jt
    block) times a free-dim broadcast of the sharded sine result.
    The LN1/q/k/v/kTv head is emitted INSIDE the panel loop (after chunk
    0's panels) so chunk 0's matmuls -> AR0 input aren't delayed by head
    PE work. gpsimd queue carries ONLY collective triggers.
  - x is shipped pre-transposed as bf16 (xT16_s); encoder + q/k/v/kTv
    matmuls are bf16-operand (f32 PSUM accum); attT/sT stay f32 (they ARE
    the output scale); FFN matmuls bf16. Measured rel err: 7.2e-3 (gate
    2e-2), HW-validated.
  - Queue plan (deadlock-free by construction): sync HWDGE = 8x 1MB
    panels, AR in-copies, then ALL collective load-backs after the last
    panel, then y. scalar HWDGE = xT16 + encoder weights FIRST (the head
    gates pass1 gates AR0), other weights, then the 8x 1MB uT prefetch
    stream (bufs=2, NOT more: deeper prefetch contends with the panel
    stream on the shared SDMA engines, delaying AR0's input and shifting
    the whole serialized collective chain — measured 39.6us/body in a
    same-session A/B, the 2nd-largest single win. pass2 only needs uT
    from ~44us and its consumption outpaces the DMA, so 2 bufs never
    starve it). PE: pass1 matmuls -> 64 utxT transposes/pass2 matmuls ->
    attention tail.
"""

import os as _os

import numpy as np

import concourse.bacc as bacc
import concourse.mybir as mybir
import concourse.tile as tile
from concourse.bass_utils import run_bass_kernel_spmd
from concourse.masks import make_identity

F32 = mybir.dt.float32
F32R = mybir.dt.float32r
F8 = mybir.dt.float8e4
BF16 = mybir.dt.bfloat16
AF = mybir.ActivationFunctionType
ALU = mybir.AluOpType

NCORES = 8
N = 8192
NFEAT = 512
HID = 256
C = 128
DIM = 32
KPOW = 10
ROWS = N // NCORES      # 1024 rows per core
NT = ROWS // 128        # 8 row tiles
JT = N // 128           # 64 column tiles
PW = 1024               # pass-1 panel width
JP = N // PW            # 16 panels
JTC = JT // NCORES      # 8 jt per core for new_e sharding
UTG = 8                 # jt tiles per uT prefetch DMA
LN_EPS = 1e-5

TWO_PI = 6.283185307179586
INV_2PI = 1.0 / TWO_PI
CW_C1 = 6.28125
CW_C2 = float(np.float32(TWO_PI - CW_C1))
CW_C3 = TWO_PI - CW_C1 - CW_C2
MAGIC = 12582912.0      # 1.5 * 2**23, round-to-nearest trick
HALF_PI = float(np.float32(np.pi / 2))
PI_F = float(np.float32(np.pi))

WEIGHT_NAMES = [
    ("fe_w1", [NFEAT, HID]), ("fe_b1", [HID]),
    ("fe_w2", [HID, C]), ("fe_b2", [C]),
    ("eig_w", [KPOW, DIM + 1]), ("eig_b", [KPOW]), ("alpha_w", [KPOW]),
    ("mha_g", [C]), ("mha_b", [C]), ("ffn_g", [C]), ("ffn_b", [C]),
    ("wq", [C, C]), ("bq", [C]), ("wk", [C, C]), ("bk", [C]),
    ("wv", [C, C]), ("bv", [C]), ("wo", [C, C]), ("bo", [C]),
    ("f1_w", [C, C]), ("f1_b", [C]), ("f2_w", [C, C]), ("f2_b", [C]),
]

# ---- packed single-input layout (byte offsets into the fp8 "pk" tensor) ----
# Per-dispatch client overhead scales with the number of I/O buffers
# (~34us/arg measured), so every input rides in ONE fp8 tensor:
#   [u8 rows | uT8 | xT16 as bytes | f32 smalls region]
OFF_U = 0
OFF_UT = ROWS * N
OFF_X = 2 * ROWS * N
OFF_SM = 2 * ROWS * N + NFEAT * ROWS * 2

_SM_ORDER = [(n, int(np.prod(s))) for n, s in WEIGHT_NAMES] + [
    ("e_js", 128 * JTC), ("ne_mask", 128 * JT),
]
SM_OFF = {}
_acc = 0
for _n, _c in _SM_ORDER:
    SM_OFF[_n] = (_acc, _c)
    _acc += _c
SM_TOTAL = _acc
PK_BYTES = OFF_SM + 4 * SM_TOTAL


def _build(nc):
    pk = nc.dram_tensor("pk", [PK_BYTES], F8, kind="ExternalInput")
    y = nc.dram_tensor("y", [ROWS, C], F32, kind="ExternalOutput")

    def smap(name):
        o, n = SM_OFF[name]
        return pk.ap()[OFF_SM + 4 * o:OFF_SM + 4 * (o + n)].bitcast(F32)

    div_const = nc.inline_tensor(
        np.tile(np.arange(1, DIM // 2 + 1, dtype=np.float32), (128, 1)), name="divc"
    )

    NAR = int(_os.environ.get("KERNEL_NAR", "2"))
    _REPL = int(_os.environ.get("KERNEL_REPLICATE", "1"))
    JPC = JP // NAR              # panels per AR chunk
    CW = N // NAR                # utxT columns per AR chunk

    with tile.TileContext(nc) as tc:
        with (
            tc.tile_pool(name="persist", bufs=1) as per,
            tc.tile_pool(name="pan", bufs=4) as pan,
            tc.tile_pool(name="u16t", bufs=2) as u16t_pool,
            tc.tile_pool(name="g16p", bufs=4) as g16_pool,
            tc.tile_pool(name="t16p", bufs=3) as t16_pool,
            tc.tile_pool(name="stats", bufs=4) as stats,
            tc.tile_pool(name="psum_sm", bufs=3, space="PSUM") as psum_sm,
            tc.tile_pool(name="psum_tr", bufs=2, space="PSUM") as psum_tr,
            tc.tile_pool(name="psum_acc", bufs=1, space="PSUM") as psum_acc,
            tc.tile_pool(name="dram", bufs=1, space="DRAM") as dram,
        ):
            def _body_once():
                rg = [list(range(NCORES))]

                # ---------------- constants / weights ----------------
                ident = per.tile([128, 128], F32, tag="ident")
                make_identity(nc, ident[:])
                ident16 = per.tile([128, 128], BF16, tag="ident16")
                make_identity(nc, ident16[:])

                eps_sb = per.tile([128, 1], F32, tag="eps_sb")
                nc.vector.memset(eps_sb[:], LN_EPS)

                div_sb = per.tile([128, DIM // 2], F32, tag="div_sb")
                nc.scalar.dma_start(out=div_sb[:], in_=div_const.ap())

                def bcast(name, width, tag):
                    t = per.tile([128, width], F32, tag=tag)
                    nc.scalar.dma_start(out=t[:], in_=smap(name).partition_broadcast(128))
                    return t

                def per_part(name, tag):
                    t = per.tile([128, 1], F32, tag=tag)
                    nc.scalar.dma_start(out=t[:], in_=smap(name).rearrange("(p o) -> p o", o=1))
                    return t

                def load16(name, shape2, tag, rearr=None):
                    """Load an f32 weight then cast to a bf16 SBUF tile."""
                    tf = per.tile(shape2, F32, tag=tag + "_f")
                    src = smap(name)
                    if rearr is None:
                        rearr = ("(p c) -> p c", dict(c=C))
                    src = src.rearrange(*rearr[:1], **rearr[1])
                    nc.scalar.dma_start(out=tf[:], in_=src)
                    tb = per.tile(shape2, BF16, tag=tag)
                    nc.vector.tensor_copy(out=tb[:], in_=tf[:])
                    return tb


                # encoder inputs FIRST on the scalar ring: the head gates
                # pass1 which gates AR0
                xT16 = per.tile([128, NFEAT // 128, ROWS], BF16, tag="xT16")
                nc.scalar.dma_start(
                    out=xT16[:],
                    in_=pk.ap()[OFF_X:OFF_X + NFEAT * ROWS * 2].bitcast(BF16)
                    .rearrange("(t p r) -> p t r", p=128, r=ROWS))
                w1_16 = load16("fe_w1", [128, NFEAT // 128, HID], "w1_16",
                               ("(t p h) -> p t h", dict(p=128, h=HID)))
                w2_16 = load16("fe_w2", [128, HID // 128, C], "w2_16",
                               ("(t p c) -> p t c", dict(p=128, c=C)))
                b1_sb = per.tile([128, HID // 128], F32, tag="b1_sb")
                nc.scalar.dma_start(out=b1_sb[:], in_=smap("fe_b1").rearrange("(t p) -> p t", p=128))
                b2_bc = bcast("fe_b2", C, "b2_bc")

                wq_16 = load16("wq", [128, C], "wq_16")
                wk_16 = load16("wk", [128, C], "wk_16")
                wv_16 = load16("wv", [128, C], "wv_16")
                wo_sb = per.tile([128, C], F32, tag="wo_sb")
                nc.scalar.dma_start(out=wo_sb[:], in_=smap("wo").rearrange("(p c) -> p c", c=C))
                f1w_16 = load16("f1_w", [128, C], "f1w_16")
                f2w_16 = load16("f2_w", [128, C], "f2w_16")

                bq_pp = per_part("bq", "bq_pp")
                bo_pp = per_part("bo", "bo_pp")
                f1b_pp = per_part("f1_b", "f1b_pp")
                f2b_pp = per_part("f2_b", "f2b_pp")
                bk_bc = bcast("bk", C, "bk_bc")
                bv_bc = bcast("bv", C, "bv_bc")
                mhag_bc = bcast("mha_g", C, "mhag_bc")
                mhab_bc = bcast("mha_b", C, "mhab_bc")
                ffng_bc = bcast("ffn_g", C, "ffng_bc")
                ffnb_bc = bcast("ffn_b", C, "ffnb_bc")


                # ---------------- feat encoder (bf16 matmuls) ----------------
                # t1^T [hid_part, 2(ht), n] = relu(w1^T x^T + b1)
                t1T16 = per.tile([128, HID // 128, ROWS], BF16, tag="t1T16")
                for ht in range(HID // 128):
                    for nch in range(ROWS // 512):
                        ps = psum_sm.tile([128, 512], F32, tag="ps_sm")
                        for ft in range(NFEAT // 128):
                            nc.tensor.matmul(
                                ps[:], lhsT=w1_16[:, ft, ht * 128:(ht + 1) * 128],
                                rhs=xT16[:, ft, nch * 512:(nch + 1) * 512],
                                start=(ft == 0), stop=(ft == NFEAT // 128 - 1),
                            )
                        nc.scalar.activation(
                            out=t1T16[:, ht, nch * 512:(nch + 1) * 512], in_=ps[:],
                            func=AF.Relu, bias=b1_sb[:, ht:ht + 1],
                        )

                # h [n_part, 8(nt), C] = t1 @ w2 + b2 (keep f32 + bf16 copies)
                h_sb = per.tile([128, NT, C], F32, tag="h_sb")
                h8_sb = per.tile([128, NT, C], F8, tag="h8_sb")
                for nt in range(NT):
                    ps = psum_sm.tile([128, C], F32, tag="ps_sm")
                    for ht in range(HID // 128):
                        nc.tensor.matmul(
                            ps[:], lhsT=t1T16[:, ht, nt * 128:(nt + 1) * 128],
                            rhs=w2_16[:, ht, :],
                            start=(ht == 0), stop=(ht == HID // 128 - 1),
                        )
                    nc.vector.tensor_add(out=h_sb[:, nt, :], in0=ps[:], in1=b2_bc[:])
                    nc.vector.tensor_copy(out=h8_sb[:, nt, :], in_=h_sb[:, nt, :])

                # ---------------- new_e (jt-sharded) + AllGather ----------------
                eigw_bc = bcast("eig_w", KPOW * (DIM + 1), "eigw_bc")
                eigb_bc = bcast("eig_b", KPOW, "eigb_bc")
                alpha_bc = bcast("alpha_w", KPOW, "alpha_bc")

                w2s = per.tile([128, KPOW, DIM // 2], F32, tag="w2s")
                w2c = per.tile([128, KPOW, DIM // 2], F32, tag="w2c")
                eigw_3d = eigw_bc[:].rearrange("p (k d) -> p k d", d=DIM + 1)
                alpha_b3 = alpha_bc[:].unsqueeze(2).broadcast_to([128, KPOW, DIM // 2])
                nc.vector.tensor_tensor(out=w2s[:], in0=alpha_b3, in1=eigw_3d[:, :, 1:1 + DIM // 2], op=ALU.mult)
                nc.vector.tensor_tensor(out=w2c[:], in0=alpha_b3, in1=eigw_3d[:, :, 1 + DIM // 2:DIM + 1], op=ALU.mult)
                w0t = per.tile([128, KPOW], F32, tag="w0t")
                nc.vector.tensor_tensor(out=w0t[:], in0=eigw_3d[:, :, 0], in1=eigb_bc[:], op=ALU.add)
                nc.vector.tensor_tensor(out=w0t[:], in0=w0t[:], in1=alpha_bc[:], op=ALU.mult)
                w0 = per.tile([128, 1], F32, tag="w0")
                nc.vector.tensor_reduce(out=w0[:], in_=w0t[:], axis=mybir.AxisListType.X, op=ALU.add)

                e_sb = per.tile([128, JTC], F32, tag="e_sb")
                nc.scalar.dma_start(out=e_sb[:], in_=smap("e_js").rearrange("(p b) -> p b", b=JTC))
                pows = per.tile([128, JTC, KPOW], F32, tag="pows")
                nc.vector.tensor_copy(out=pows[:, :, 0], in_=e_sb[:])
                for k in range(1, KPOW):
                    nc.vector.tensor_tensor(out=pows[:, :, k], in0=pows[:, :, k - 1], in1=e_sb[:], op=ALU.mult)

                WNE = JTC * KPOW * (DIM // 2)  # 1280
                pe_t = per.tile([128, JTC, KPOW, DIM // 2], F32, tag="pe_t")
                kq_t = per.tile([128, WNE], F32, tag="kq_t")
                trig = per.tile([128, WNE], F32, tag="trig")
                ne_s = per.tile([128, JTC], F32, tag="ne_s")
                ne_c = per.tile([128, JTC], F32, tag="ne_c")

                pows_b = pows[:].unsqueeze(3).broadcast_to([128, JTC, KPOW, DIM // 2])
                div_b = div_sb[:].unsqueeze(1).unsqueeze(1).broadcast_to([128, JTC, KPOW, DIM // 2])
                nc.vector.tensor_tensor(out=pe_t[:], in0=pows_b, in1=div_b, op=ALU.mult)
                pe_f = pe_t[:].rearrange("p a b c -> p (a b c)")
                nc.vector.tensor_scalar(out=kq_t[:], in0=pe_f, scalar1=INV_2PI, scalar2=MAGIC, op0=ALU.mult, op1=ALU.add)
                nc.vector.tensor_scalar_sub(out=kq_t[:], in0=kq_t[:], scalar1=MAGIC)
                nc.vector.cody_waite_cascade(pe_f, pe_f, kq_t[:], CW_C1, CW_C2, CW_C3)

                w2s_b = w2s[:].rearrange("p k d -> p (k d)").unsqueeze(1).broadcast_to([128, JTC, KPOW * DIM // 2])
                w2c_b = w2c[:].rearrange("p k d -> p (k d)").unsqueeze(1).broadcast_to([128, JTC, KPOW * DIM // 2])

                nc.scalar.activation(out=trig[:], in_=pe_f, func=AF.Sin)
                trig3 = trig[:].rearrange("p (a w) -> p a w", a=JTC)
                nc.vector.tensor_tensor(out=trig3, in0=trig3, in1=w2s_b, op=ALU.mult)
                nc.vector.tensor_reduce(out=ne_s[:], in_=trig3, axis=mybir.AxisListType.X, op=ALU.add)

                nc.vector.add_range_wrap(kq_t[:], pe_f, HALF_PI, PI_F, TWO_PI)
                nc.scalar.activation(out=trig[:], in_=kq_t[:], func=AF.Sin)
                nc.vector.tensor_tensor(out=trig3, in0=trig3, in1=w2c_b, op=ALU.mult)
                nc.vector.tensor_reduce(out=ne_c[:], in_=trig3, axis=mybir.AxisListType.X, op=ALU.add)

                nc.vector.tensor_tensor(out=ne_s[:], in0=ne_s[:], in1=ne_c[:], op=ALU.add)
                nc.vector.tensor_scalar_add(out=ne_s[:], in0=ne_s[:], scalar1=w0[:])

                # ne placed into fp8 via host mask (16.0 on own jt block):
                # AllGather == AllReduce of zero-padded buffers
                mask_sb = per.tile([128, JT], F32, tag="mask_sb")
                nc.scalar.dma_start(out=mask_sb[:], in_=smap("ne_mask").rearrange("(p j) -> p j", j=JT))
                ne8_placed = per.tile([128, JT], F8, tag="ne8_placed")
                ne_bc3 = ne_s[:].unsqueeze(1).broadcast_to([128, NCORES, JTC])
                nc.vector.tensor_tensor(
                    out=ne8_placed[:].rearrange("p (r b) -> p r b", b=JTC),
                    in0=ne_bc3,
                    in1=mask_sb[:].rearrange("p (r b) -> p r b", b=JTC),
                    op=ALU.mult)
                ne8_lb = per.tile([128, JT], F8, tag="ne8_lb")
                new_e_sb = per.tile([128, JT], F32, tag="new_e_sb")

                # ---------------- LN1 + q/k/v + kTv partial ----------------
                def layer_norm(src, dst, g_bc, b_bc):
                    for nt in range(NT):
                        st = stats.tile([128, 6], F32, tag="ln_st")
                        nc.vector.bn_stats(out=st[:], in_=src[:, nt, :])
                        mv = stats.tile([128, 2], F32, tag="ln_mv")
                        nc.vector.bn_aggr(out=mv[:], in_=st[:])
                        rstd = stats.tile([128, 1], F32, tag="ln_rstd")
                        nc.scalar.activation(out=rstd[:], in_=mv[:, 1:2], func=AF.Sqrt, bias=eps_sb[:])
                        nc.vector.reciprocal(out=rstd[:], in_=rstd[:])
                        nc.vector.tensor_scalar(
                            out=dst[:, nt, :], in0=src[:, nt, :],
                            scalar1=mv[:, 0:1], scalar2=rstd[:],
                            op0=ALU.subtract, op1=ALU.mult,
                        )
                        nc.vector.tensor_tensor(out=dst[:, nt, :], in0=dst[:, nt, :], in1=g_bc[:], op=ALU.mult)
                        nc.vector.tensor_tensor(out=dst[:, nt, :], in0=dst[:, nt, :], in1=b_bc[:], op=ALU.add)

                mh_sb = per.tile([128, NT, C], F32, tag="mh_sb")

                mh16T = per.tile([128, ROWS], BF16, tag="mh16T")
                qT = per.tile([128, ROWS], F32, tag="qT")
                k16_sb = per.tile([128, NT, C], BF16, tag="k16_sb")
                v16_sb = per.tile([128, NT, C], BF16, tag="v16_sb")
                kTv_sb = per.tile([128, C], F32, tag="kTv_sb")

                def emit_qkv_head():
                    layer_norm(h_sb, mh_sb, mhag_bc, mhab_bc)
                    for nt in range(NT):
                        tp = psum_tr.tile([128, 128], F32, tag="tr")
                        nc.tensor.transpose(tp[:], mh_sb[:, nt, :], ident[:])
                        nc.vector.tensor_copy(out=mh16T[:, nt * 128:(nt + 1) * 128], in_=tp[:])
                    for nch in range(ROWS // 512):
                        ps = psum_sm.tile([128, 512], F32, tag="ps_sm")
                        nc.tensor.matmul(ps[:], lhsT=wq_16[:], rhs=mh16T[:, nch * 512:(nch + 1) * 512], start=True, stop=True)
                        nc.scalar.activation(out=qT[:, nch * 512:(nch + 1) * 512], in_=ps[:], func=AF.Identity, bias=bq_pp[:])
                    for nt in range(NT):
                        ps = psum_sm.tile([128, C], F32, tag="ps_sm")
                        nc.tensor.matmul(ps[:], lhsT=mh16T[:, nt * 128:(nt + 1) * 128], rhs=wk_16[:], start=True, stop=True)
                        nc.vector.tensor_add(out=k16_sb[:, nt, :], in0=ps[:], in1=bk_bc[:])
                        ps2 = psum_sm.tile([128, C], F32, tag="ps_sm")
                        nc.tensor.matmul(ps2[:], lhsT=mh16T[:, nt * 128:(nt + 1) * 128], rhs=wv_16[:], start=True, stop=True)
                        nc.vector.tensor_add(out=v16_sb[:, nt, :], in0=ps2[:], in1=bv_bc[:])
                    pskv = psum_sm.tile([128, C], F32, tag="ps_sm")
                    for nt in range(NT):
                        nc.tensor.matmul(pskv[:], lhsT=k16_sb[:, nt, :], rhs=v16_sb[:, nt, :], start=(nt == 0), stop=(nt == NT - 1))
                    nc.vector.tensor_copy(out=kTv_sb[:], in_=pskv[:])

                # ---------------- collectives (triggers on gpsimd only) ----------------
                utxT = per.tile([128, N], F8, tag="utxT")
                ar_ins, ar_outs = [], []
                for c in range(NAR):
                    w = CW + (JT if c == 0 else 0)  # chunk 0 carries ne columns
                    ari = dram.tile([128, w], F8, tag=f"ar_in{c}", name=f"ar_in{c}")
                    aro = dram.tile([128, w], F8, tag=f"ar_out{c}", name=f"ar_out{c}",
                                    addr_space="Shared")
                    ar_ins.append(ari)
                    ar_outs.append(aro)
                ktv_in = dram.tile([128, C], F32, tag="ktv_in")
                ktv_out = dram.tile([128, C], F32, tag="ktv_out", addr_space="Shared")

                def emit_ktv_trigger():
                    nc.sync.dma_start(out=ktv_in[:], in_=kTv_sb[:])
                    nc.gpsimd.collective_compute(
                        "AllReduce", ALU.add, replica_groups=rg,
                        ins=[ktv_in[:].opt()], outs=[ktv_out[:].opt()],
                    )

                def emit_ar_trigger(c):
                    nc.sync.dma_start(out=ar_ins[c][:, 0:CW], in_=utxT[:, c * CW:(c + 1) * CW])
                    if c == 0:
                        nc.sync.dma_start(out=ar_ins[0][:, CW:CW + JT], in_=ne8_placed[:])
                    nc.gpsimd.collective_compute(
                        "AllReduce", ALU.add, replica_groups=rg,
                        ins=[ar_ins[c][:].opt()], outs=[ar_outs[c][:].opt()],
                    )

                # ---------------- pass 1: utx^T = h16^T @ u panels ----------------
                u_r = pk.ap()[OFF_U:OFF_U + ROWS * N].rearrange("(t p j) -> p t j", p=128, j=N)
                for jp in range(JP):
                    panel = pan.tile([128, NT, PW], F8, tag="panel")
                    nc.sync.dma_start(out=panel[:], in_=u_r[:, :, jp * PW:(jp + 1) * PW])
                    if jp == min(JPC, JP - 1):
                        emit_qkv_head()
                    if jp >= JPC + 1 and (jp - JPC - 1) % JPC == 0 and (jp - JPC - 1) // JPC < NAR - 1:
                        emit_ar_trigger((jp - JPC - 1) // JPC)
                    for jh in range(PW // 512):
                        ps = psum_sm.tile([128, 512], F32, tag="ps_sm")
                        for ntp in range(NT // 2):
                            nc.tensor.matmul(
                                ps[:], lhsT=h8_sb[:, 2 * ntp:2 * ntp + 2, :],
                                rhs=panel[:, 2 * ntp:2 * ntp + 2, jh * 512:(jh + 1) * 512],
                                start=(ntp == 0), stop=(ntp == NT // 2 - 1),
                                perf_mode=mybir.MatmulPerfMode.DoubleRow,
                            )
                        nc.scalar.activation(
                            out=utxT[:, jp * PW + jh * 512:jp * PW + (jh + 1) * 512],
                            in_=ps[:], func=AF.Identity)
                emit_ar_trigger(NAR - 1)
                emit_ktv_trigger()

                # all collective load-backs, in completion order, after the last
                # panel so they never block the panel stream's FIFO
                nc.sync.dma_start(out=utxT[:, 0:CW], in_=ar_outs[0][:, 0:CW])
                nc.sync.dma_start(out=ne8_lb[:], in_=ar_outs[0][:, CW:CW + JT])
                nc.vector.tensor_copy(out=new_e_sb[:], in_=ne8_lb[:])
                for c in range(1, NAR):
                    nc.sync.dma_start(out=utxT[:, c * CW:(c + 1) * CW], in_=ar_outs[c][:])
                nc.sync.dma_start(out=kTv_sb[:], in_=ktv_out[:])

                # ---------------- pass 2: h_fur^T = sum_jt g16[jt]^T @ uT16[jt] ----------------
                uT_r = pk.ap()[OFF_UT:OFF_UT + N * ROWS].rearrange("(jt p r) -> p jt r", p=128, r=ROWS)
                hfur_ps = psum_acc.tile([128, ROWS], F32, tag="hfur")
                for jtg in range(JT // UTG):
                    ut = u16t_pool.tile([128, UTG, ROWS], F8, tag="ut")
                    nc.scalar.dma_start(out=ut[:], in_=uT_r[:, jtg * UTG:(jtg + 1) * UTG, :])
                    for jpr in range(UTG // 2):
                        g8p = g16_pool.tile([128, 2, 128], F8, tag="g8p")
                        for k in range(2):
                            jt = jtg * UTG + jpr * 2 + k
                            t16 = t16_pool.tile([128, 128], BF16, tag="t16")
                            nc.vector.tensor_copy(out=t16[:], in_=utxT[:, jt * 128:(jt + 1) * 128])
                            tp = psum_tr.tile([128, 128], BF16, tag="tr", name="tp16")
                            nc.tensor.transpose(tp[:], t16[:], ident16[:])
                            nc.vector.tensor_scalar_mul(out=g8p[:, k, :], in0=tp[:], scalar1=new_e_sb[:, jt:jt + 1])
                        pair = jtg * (UTG // 2) + jpr
                        for hf in range(ROWS // 512):
                            nc.tensor.matmul(
                                hfur_ps[:, hf * 512:(hf + 1) * 512], lhsT=g8p[:],
                                rhs=ut[:, jpr * 2:jpr * 2 + 2, hf * 512:(hf + 1) * 512],
                                start=(pair == 0), stop=(pair == JT // 2 - 1),
                                skip_group_check=True,
                                perf_mode=mybir.MatmulPerfMode.DoubleRow,
                            )

                # ---------------- att^T (fp32r), s^T, h1 ----------------
                hfurT = per.tile([128, ROWS], F32, tag="hfurT")
                nc.vector.tensor_scalar_mul(out=hfurT[:], in0=hfur_ps[:], scalar1=1.0 / 4096.0)

                attT = per.tile([128, ROWS], F32, tag="attT")
                for nch in range(ROWS // 512):
                    ps = psum_sm.tile([128, 512], F32, tag="ps_sm")
                    nc.tensor.matmul(ps[:], lhsT=kTv_sb[:],
                                     rhs=qT[:, nch * 512:(nch + 1) * 512],
                                     start=True, stop=True)
                    nc.vector.tensor_copy(out=attT[:, nch * 512:(nch + 1) * 512], in_=ps[:])

                sT = per.tile([128, ROWS], F32, tag="sT")
                for nch in range(ROWS // 512):
                    ps = psum_sm.tile([128, 512], F32, tag="ps_sm")
                    nc.tensor.matmul(ps[:], lhsT=wo_sb[:],
                                     rhs=attT[:, nch * 512:(nch + 1) * 512],
                                     start=True, stop=True)
                    nc.vector.scalar_tensor_tensor(
                        out=sT[:, nch * 512:(nch + 1) * 512], in0=ps[:], scalar=bo_pp[:],
                        in1=hfurT[:, nch * 512:(nch + 1) * 512],
                        op0=ALU.add, op1=ALU.add,
                    )

                h1_sb = per.tile([128, NT, C], F32, tag="h1_sb")
                for nt in range(NT):
                    tp = psum_tr.tile([128, 128], F32, tag="tr")
                    nc.tensor.transpose(tp[:], sT[:, nt * 128:(nt + 1) * 128], ident[:])
                    nc.vector.tensor_add(out=h1_sb[:, nt, :], in0=tp[:], in1=h_sb[:, nt, :])

                # ---------------- FFN ----------------
                mh2_sb = per.tile([128, NT, C], F32, tag="mh2_sb")
                layer_norm(h1_sb, mh2_sb, ffng_bc, ffnb_bc)
                mh2T = per.tile([128, ROWS], BF16, tag="mh2T")
                for nt in range(NT):
                    tp = psum_tr.tile([128, 128], F32, tag="tr")
                    nc.tensor.transpose(tp[:], mh2_sb[:, nt, :], ident[:])
                    nc.vector.tensor_copy(out=mh2T[:, nt * 128:(nt + 1) * 128], in_=tp[:])

                gzT = per.tile([128, ROWS], BF16, tag="gzT")
                for nch in range(ROWS // 512):
                    ps = psum_sm.tile([128, 512], F32, tag="ps_sm")
                    nc.tensor.matmul(ps[:], lhsT=f1w_16[:],
                                     rhs=mh2T[:, nch * 512:(nch + 1) * 512],
                                     start=True, stop=True)
                    nc.scalar.activation(out=gzT[:, nch * 512:(nch + 1) * 512], in_=ps[:], func=AF.Gelu, bias=f1b_pp[:])

                f2T = per.tile([128, ROWS], F32, tag="f2T")
                for nch in range(ROWS // 512):
                    ps = psum_sm.tile([128, 512], F32, tag="ps_sm")
                    nc.tensor.matmul(ps[:], lhsT=f2w_16[:],
                                     rhs=gzT[:, nch * 512:(nch + 1) * 512],
                                     start=True, stop=True)
                    nc.scalar.activation(out=f2T[:, nch * 512:(nch + 1) * 512], in_=ps[:], func=AF.Identity, bias=f2b_pp[:])

                hout_sb = per.tile([128, NT, C], F32, tag="hout_sb")
                for nt in range(NT):
                    tp = psum_tr.tile([128, 128], F32, tag="tr")
                    nc.tensor.transpose(tp[:], f2T[:, nt * 128:(nt + 1) * 128], ident[:])
                    nc.vector.tensor_add(out=hout_sb[:, nt, :], in0=tp[:], in1=h1_sb[:, nt, :])

                nc.sync.dma_start(out=y.ap().rearrange("(t p) c -> p t c", p=128), in_=hout_sb[:])

            for _rep in range(_REPL):
                _body_once()

    nc.compile()
    return nc


_NC = None


def _get_nc():
    global _NC
    if _NC is None:
        _NC = _build(bacc.Bacc("TRN2", target_bir_lowering=False, debug=False, num_devices=NCORES))
    return _NC


def make_in_maps(inputs):
    import ml_dtypes
    BF = ml_dtypes.bfloat16
    F8E4 = ml_dtypes.float8_e4m3

    e = np.ascontiguousarray(np.asarray(inputs["e"], dtype=np.float32))
    u = np.asarray(inputs["u"], dtype=np.float32)
    x = np.asarray(inputs["x"], dtype=np.float32)
    e_resh = np.ascontiguousarray(e.reshape(JT, 128).T)  # [p, jt] = e[jt*128+p]

    sm_common = np.zeros(SM_TOTAL, np.float32)
    for name, _ in WEIGHT_NAMES:
        o, n = SM_OFF[name]
        sm_common[o:o + n] = np.asarray(inputs[name], dtype=np.float32).ravel()

    in_maps = []
    for m in range(NCORES):
        us = u[m * ROWS:(m + 1) * ROWS]
        xs = x[m * ROWS:(m + 1) * ROWS]
        us64 = us * np.float32(16.0)

        sm = sm_common.copy()
        o, n = SM_OFF["e_js"]
        sm[o:o + n] = np.ascontiguousarray(e_resh[:, m * JTC:(m + 1) * JTC]).ravel()
        mask = np.zeros((128, JT), np.float32)
        mask[:, m * JTC:(m + 1) * JTC] = 16.0
        o, n = SM_OFF["ne_mask"]
        sm[o:o + n] = mask.ravel()

        buf = np.empty(PK_BYTES, np.uint8)
        buf[OFF_U:OFF_U + ROWS * N] = us64.astype(F8E4).view(np.uint8).ravel()
        buf[OFF_UT:OFF_UT + N * ROWS] = np.ascontiguousarray(us64.T).astype(F8E4).view(np.uint8).ravel()
        buf[OFF_X:OFF_X + NFEAT * ROWS * 2] = np.ascontiguousarray(xs.T).astype(BF).view(np.uint8).ravel()
        buf[OFF_SM:] = sm.view(np.uint8)
        in_maps.append({"pk": buf.view(F8E4)})
    return in_maps


def kernel(**inputs):
    nc = _get_nc()
    in_maps = make_in_maps(inputs)

    trace = bool(int(_os.environ.get("KERNEL_TRACE", "0")))
    res = run_bass_kernel_spmd(nc, in_maps, core_ids=list(range(NCORES)), trace=trace)
    if trace and res.exec_time_ns is not None:
        print(f"HW exec time: {res.exec_time_ns} ns")
        if res.instructions_and_trace is not None:
            print("trace:", res.instructions_and_trace[1])
    out = np.concatenate([r["y"] for r in res.results], axis=0)
    return out.astype(np.float32)



# revision 12
# speedup vs baseline: 29.2530x; 2.5973x over previous
"""TRN2 Bass/Tile kernel for nn_NoFoDifformer (8-core SPMD, row-sharded).

v10 design (lineage: v1 ~374us / v6 ~127 / v8 ~122 / v9 ~114 / v10 ~75us):
  - u is shipped from the host in BOTH layouts as fp8e4m3 (IEEE variant,
    max 240!), host-scaled x16 to sit in fp8 range: u8_s [ROWS,N] (row
    slice, pass 1) and uT8_s [N,ROWS] (pre-transposed, pass 2). This kills
    the DRAM scratch round-trip, DMA-transpose reads, the SBUF stash + 512
    PE transposes and SWDGE cast-loads of v1, and halves u HBM traffic to
    16MB/core of plain HWDGE streaming. The output is attention-dominated
    (h_fur absmax 0.12 vs output 3.3e4) so u-path precision is free.
  - pass1/pass2 matmuls run fp8 DoubleRow (2 contraction blocks per
    instruction, exact arithmetic, PE time halved to ~49us modeled).
  - utx is AllReduced in fp8 (utx16 absmax 128 < 240): NAR chunks of
    0.25MB ride the panel stream; g = new_e*utx carries the second x16;
    1/256 is folded into the h_fur psum copy. utxT tiles stage through a
    bf16 copy before PE transpose (the verifier rejects fp8 transpose
    outputs to PSUM).
  - Collective SPAD order matters (ncfw executes in program order, each
    with a ~10us floor): AR0, AR1, kTv last (consumed post-pass2). NAR=2
    beats 4 (fewer floors). The new_e AllGather is ELIMINATED: AllGather ==
    AllReduce of zero-padded buffers, so 16*ne (fp8) rides AR0 as JT extra
    columns, placed via a host-provided per-core mask (16.0 on own jt
    block) times a free-dim broadcast of the sharded sine result.
    The LN1/q/k/v/kTv head is emitted INSIDE the panel loop (after chunk
    0's panels) so chunk 0's matmuls -> AR0 input aren't delayed by head
    PE work. gpsimd queue carries ONLY collective triggers.
  - x is shipped pre-transposed as bf16 (xT16_s); encoder + q/k/v/kTv
    matmuls are bf16-operand (f32 PSUM accum); attT/sT stay f32 (they ARE
    the output scale); FFN matmuls bf16. Measured rel err: 7.2e-3 (gate
    2e-2), HW-validated.
  - Queue plan (deadlock-free by construction): sync HWDGE = 8x 1MB
    panels, AR in-copies, then ALL collective load-backs after the last
    panel, then y. scalar HWDGE = xT16 + encoder weights FIRST (the head
    gates pass1 gates AR0), other weights, then the 8x 1MB uT prefetch
    stream (bufs=2, NOT more: deeper prefetch contends with the panel
    stream on the shared SDMA engines, delaying AR0's input and shifting
    the whole serialized collective chain — measured 39.6us/body in a
    same-session A/B, the 2nd-largest single win. pass2 only needs uT
    from ~44us and its consumption outpaces the DMA, so 2 bufs never
    starve it). PE: pass1 matmuls -> 64 utxT transposes/pass2 matmuls ->
    attention tail.
"""

import os as _os

import numpy as np

import concourse.bacc as bacc
import concourse.mybir as mybir
import concourse.tile as tile
from concourse.bass_utils import run_bass_kernel_spmd
from concourse.masks import make_identity

F32 = mybir.dt.float32
F32R = mybir.dt.float32r
F8 = mybir.dt.float8e4
BF16 = mybir.dt.bfloat16
AF = mybir.ActivationFunctionType
ALU = mybir.AluOpType

NCORES = 8
N = 8192
NFEAT = 512
HID = 256
C = 128
DIM = 32
KPOW = 10
ROWS = N // NCORES      # 1024 rows per core
NT = ROWS // 128        # 8 row tiles
JT = N // 128           # 64 column tiles
PW = 1024               # pass-1 panel width
JP = N // PW            # 16 panels
JTC = JT // NCORES      # 8 jt per core for new_e sharding
UTG = 8                 # jt tiles per uT prefetch DMA
LN_EPS = 1e-5

TWO_PI = 6.283185307179586
INV_2PI = 1.0 / TWO_PI
CW_C1 = 6.28125
CW_C2 = float(np.float32(TWO_PI - CW_C1))
CW_C3 = TWO_PI - CW_C1 - CW_C2
MAGIC = 12582912.0      # 1.5 * 2**23, round-to-nearest trick
HALF_PI = float(np.float32(np.pi / 2))
PI_F = float(np.float32(np.pi))

WEIGHT_NAMES = [
    ("fe_w1", [NFEAT, HID]), ("fe_b1", [HID]),
    ("fe_w2", [HID, C]), ("fe_b2", [C]),
    ("eig_w", [KPOW, DIM + 1]), ("eig_b", [KPOW]), ("alpha_w", [KPOW]),
    ("mha_g", [C]), ("mha_b", [C]), ("ffn_g", [C]), ("ffn_b", [C]),
    ("wq", [C, C]), ("bq", [C]), ("wk", [C, C]), ("bk", [C]),
    ("wv", [C, C]), ("bv", [C]), ("wo", [C, C]), ("bo", [C]),
    ("f1_w", [C, C]), ("f1_b", [C]), ("f2_w", [C, C]), ("f2_b", [C]),
]

# ---- packed single-input layout (byte offsets into the fp8 "pk" tensor) ----
# Per-dispatch client overhead scales with the number of I/O buffers
# (~34us/arg measured), so every input rides in ONE fp8 tensor:
#   [u8 rows | uT8 | xT16 as bytes | f32 smalls region]
OFF_U = 0
OFF_UT = ROWS * N
OFF_X = 2 * ROWS * N
OFF_SM = 2 * ROWS * N + NFEAT * ROWS * 2

_SM_ORDER = [(n, int(np.prod(s))) for n, s in WEIGHT_NAMES] + [
    ("e_js", 128 * JTC), ("ne_mask", 128 * JT),
]
SM_OFF = {}
_acc = 0
for _n, _c in _SM_ORDER:
    SM_OFF[_n] = (_acc, _c)
    _acc += _c
SM_TOTAL = _acc
PK_BYTES = OFF_SM + 4 * SM_TOTAL


def _build(nc):
    pk = nc.dram_tensor("pk", [PK_BYTES], F8, kind="ExternalInput")
    y = nc.dram_tensor("y", [ROWS, C], F32, kind="ExternalOutput")

    def smap(name):
        o, n = SM_OFF[name]
        return pk.ap()[OFF_SM + 4 * o:OFF_SM + 4 * (o + n)].bitcast(F32)

    div_const = nc.inline_tensor(
        np.tile(np.arange(1, DIM // 2 + 1, dtype=np.float32), (128, 1)), name="divc"
    )

    NAR = int(_os.environ.get("KERNEL_NAR", "2"))
    _REPL = int(_os.environ.get("KERNEL_REPLICATE", "1"))
    _ABL = set(f for f in _os.environ.get("KERNEL_ABLATE", "").split(",") if f)
    JPC = JP // NAR              # panels per AR chunk
    CW = N // NAR                # utxT columns per AR chunk

    with tile.TileContext(nc) as tc:
        with (
            tc.tile_pool(name="persist", bufs=1) as per,
            tc.tile_pool(name="pan", bufs=4) as pan,
            tc.tile_pool(name="u16t", bufs=2) as u16t_pool,
            tc.tile_pool(name="g16p", bufs=4) as g16_pool,
            tc.tile_pool(name="t16p", bufs=3) as t16_pool,
            tc.tile_pool(name="stats", bufs=4) as stats,
            tc.tile_pool(name="psum_sm", bufs=3, space="PSUM") as psum_sm,
            tc.tile_pool(name="psum_tr", bufs=2, space="PSUM") as psum_tr,
            tc.tile_pool(name="psum_acc", bufs=1, space="PSUM") as psum_acc,
            tc.tile_pool(name="dram", bufs=1, space="DRAM") as dram,
        ):
            def _body_once():
                rg = [list(range(NCORES))]

                # ---------------- constants / weights ----------------
                ident = per.tile([128, 128], F32, tag="ident")
                make_identity(nc, ident[:])
                ident16 = per.tile([128, 128], BF16, tag="ident16")
                make_identity(nc, ident16[:])

                eps_sb = per.tile([128, 1], F32, tag="eps_sb")
                nc.vector.memset(eps_sb[:], LN_EPS)

                div_sb = per.tile([128, DIM // 2], F32, tag="div_sb")
                nc.scalar.dma_start(out=div_sb[:], in_=div_const.ap())

                def bcast(name, width, tag):
                    t = per.tile([128, width], F32, tag=tag)
                    nc.scalar.dma_start(out=t[:], in_=smap(name).partition_broadcast(128))
                    return t

                def per_part(name, tag):
                    t = per.tile([128, 1], F32, tag=tag)
                    nc.scalar.dma_start(out=t[:], in_=smap(name).rearrange("(p o) -> p o", o=1))
                    return t

                def load16(name, shape2, tag, rearr=None):
                    """Load an f32 weight then cast to a bf16 SBUF tile."""
                    tf = per.tile(shape2, F32, tag=tag + "_f")
                    src = smap(name)
                    if rearr is None:
                        rearr = ("(p c) -> p c", dict(c=C))
                    src = src.rearrange(*rearr[:1], **rearr[1])
                    nc.scalar.dma_start(out=tf[:], in_=src)
                    tb = per.tile(shape2, BF16, tag=tag)
                    nc.vector.tensor_copy(out=tb[:], in_=tf[:])
                    return tb


                # encoder inputs FIRST on the scalar ring: the head gates
                # pass1 which gates AR0
                xT16 = per.tile([128, NFEAT // 128, ROWS], BF16, tag="xT16")
                nc.scalar.dma_start(
                    out=xT16[:],
                    in_=pk.ap()[OFF_X:OFF_X + NFEAT * ROWS * 2].bitcast(BF16)
                    .rearrange("(t p r) -> p t r", p=128, r=ROWS))
                w1_16 = load16("fe_w1", [128, NFEAT // 128, HID], "w1_16",
                               ("(t p h) -> p t h", dict(p=128, h=HID)))
                w2_16 = load16("fe_w2", [128, HID // 128, C], "w2_16",
                               ("(t p c) -> p t c", dict(p=128, c=C)))
                b1_sb = per.tile([128, HID // 128], F32, tag="b1_sb")
                nc.scalar.dma_start(out=b1_sb[:], in_=smap("fe_b1").rearrange("(t p) -> p t", p=128))
                b2_bc = bcast("fe_b2", C, "b2_bc")

                wq_16 = load16("wq", [128, C], "wq_16")
                wk_16 = load16("wk", [128, C], "wk_16")
                wv_16 = load16("wv", [128, C], "wv_16")
                wo_sb = per.tile([128, C], F32, tag="wo_sb")
                nc.scalar.dma_start(out=wo_sb[:], in_=smap("wo").rearrange("(p c) -> p c", c=C))
                f1w_16 = load16("f1_w", [128, C], "f1w_16")
                f2w_16 = load16("f2_w", [128, C], "f2w_16")

                bq_pp = per_part("bq", "bq_pp")
                bo_pp = per_part("bo", "bo_pp")
                f1b_pp = per_part("f1_b", "f1b_pp")
                f2b_pp = per_part("f2_b", "f2b_pp")
                bk_bc = bcast("bk", C, "bk_bc")
                bv_bc = bcast("bv", C, "bv_bc")
                mhag_bc = bcast("mha_g", C, "mhag_bc")
                mhab_bc = bcast("mha_b", C, "mhab_bc")
                ffng_bc = bcast("ffn_g", C, "ffng_bc")
                ffnb_bc = bcast("ffn_b", C, "ffnb_bc")


                # ---------------- feat encoder (bf16 matmuls) ----------------
                # t1^T [hid_part, 2(ht), n] = relu(w1^T x^T + b1)
                t1T16 = per.tile([128, HID // 128, ROWS], BF16, tag="t1T16")
                for ht in range(HID // 128):
                    for nch in range(ROWS // 512):
                        ps = psum_sm.tile([128, 512], F32, tag="ps_sm")
                        for ft in range(NFEAT // 128):
                            nc.tensor.matmul(
                                ps[:], lhsT=w1_16[:, ft, ht * 128:(ht + 1) * 128],
                                rhs=xT16[:, ft, nch * 512:(nch + 1) * 512],
                                start=(ft == 0), stop=(ft == NFEAT // 128 - 1),
                            )
                        nc.scalar.activation(
                            out=t1T16[:, ht, nch * 512:(nch + 1) * 512], in_=ps[:],
                            func=AF.Relu, bias=b1_sb[:, ht:ht + 1],
                        )

                # h [n_part, 8(nt), C] = t1 @ w2 + b2 (keep f32 + bf16 copies)
                h_sb = per.tile([128, NT, C], F32, tag="h_sb")
                h8_sb = per.tile([128, NT, C], F8, tag="h8_sb")
                for nt in range(NT):
                    ps = psum_sm.tile([128, C], F32, tag="ps_sm")
                    for ht in range(HID // 128):
                        nc.tensor.matmul(
                            ps[:], lhsT=t1T16[:, ht, nt * 128:(nt + 1) * 128],
                            rhs=w2_16[:, ht, :],
                            start=(ht == 0), stop=(ht == HID // 128 - 1),
                        )
                    nc.vector.tensor_add(out=h_sb[:, nt, :], in0=ps[:], in1=b2_bc[:])
                    nc.vector.tensor_copy(out=h8_sb[:, nt, :], in_=h_sb[:, nt, :])

                # ---------------- new_e (jt-sharded) + AllGather ----------------
                eigw_bc = bcast("eig_w", KPOW * (DIM + 1), "eigw_bc")
                eigb_bc = bcast("eig_b", KPOW, "eigb_bc")
                alpha_bc = bcast("alpha_w", KPOW, "alpha_bc")

                w2s = per.tile([128, KPOW, DIM // 2], F32, tag="w2s")
                w2c = per.tile([128, KPOW, DIM // 2], F32, tag="w2c")
                eigw_3d = eigw_bc[:].rearrange("p (k d) -> p k d", d=DIM + 1)
                alpha_b3 = alpha_bc[:].unsqueeze(2).broadcast_to([128, KPOW, DIM // 2])
                nc.vector.tensor_tensor(out=w2s[:], in0=alpha_b3, in1=eigw_3d[:, :, 1:1 + DIM // 2], op=ALU.mult)
                nc.vector.tensor_tensor(out=w2c[:], in0=alpha_b3, in1=eigw_3d[:, :, 1 + DIM // 2:DIM + 1], op=ALU.mult)
                w0t = per.tile([128, KPOW], F32, tag="w0t")
                nc.vector.tensor_tensor(out=w0t[:], in0=eigw_3d[:, :, 0], in1=eigb_bc[:], op=ALU.add)
                nc.vector.tensor_tensor(out=w0t[:], in0=w0t[:], in1=alpha_bc[:], op=ALU.mult)
                w0 = per.tile([128, 1], F32, tag="w0")
                nc.vector.tensor_reduce(out=w0[:], in_=w0t[:], axis=mybir.AxisListType.X, op=ALU.add)

                e_sb = per.tile([128, JTC], F32, tag="e_sb")
                nc.scalar.dma_start(out=e_sb[:], in_=smap("e_js").rearrange("(p b) -> p b", b=JTC))
                pows = per.tile([128, JTC, KPOW], F32, tag="pows")
                nc.vector.tensor_copy(out=pows[:, :, 0], in_=e_sb[:])
                for k in range(1, KPOW):
                    nc.vector.tensor_tensor(out=pows[:, :, k], in0=pows[:, :, k - 1], in1=e_sb[:], op=ALU.mult)

                WNE = JTC * KPOW * (DIM // 2)  # 1280
                pe_t = per.tile([128, JTC, KPOW, DIM // 2], F32, tag="pe_t")
                kq_t = per.tile([128, WNE], F32, tag="kq_t")
                trig = per.tile([128, WNE], F32, tag="trig")
                ne_s = per.tile([128, JTC], F32, tag="ne_s")
                ne_c = per.tile([128, JTC], F32, tag="ne_c")

                pows_b = pows[:].unsqueeze(3).broadcast_to([128, JTC, KPOW, DIM // 2])
                div_b = div_sb[:].unsqueeze(1).unsqueeze(1).broadcast_to([128, JTC, KPOW, DIM // 2])
                nc.vector.tensor_tensor(out=pe_t[:], in0=pows_b, in1=div_b, op=ALU.mult)
                pe_f = pe_t[:].rearrange("p a b c -> p (a b c)")
                nc.vector.tensor_scalar(out=kq_t[:], in0=pe_f, scalar1=INV_2PI, scalar2=MAGIC, op0=ALU.mult, op1=ALU.add)
                nc.vector.tensor_scalar_sub(out=kq_t[:], in0=kq_t[:], scalar1=MAGIC)
                nc.vector.cody_waite_cascade(pe_f, pe_f, kq_t[:], CW_C1, CW_C2, CW_C3)

                w2s_b = w2s[:].rearrange("p k d -> p (k d)").unsqueeze(1).broadcast_to([128, JTC, KPOW * DIM // 2])
                w2c_b = w2c[:].rearrange("p k d -> p (k d)").unsqueeze(1).broadcast_to([128, JTC, KPOW * DIM // 2])

                nc.scalar.activation(out=trig[:], in_=pe_f, func=AF.Sin)
                trig3 = trig[:].rearrange("p (a w) -> p a w", a=JTC)
                nc.vector.tensor_tensor(out=trig3, in0=trig3, in1=w2s_b, op=ALU.mult)
                nc.vector.tensor_reduce(out=ne_s[:], in_=trig3, axis=mybir.AxisListType.X, op=ALU.add)

                nc.vector.add_range_wrap(kq_t[:], pe_f, HALF_PI, PI_F, TWO_PI)
                nc.scalar.activation(out=trig[:], in_=kq_t[:], func=AF.Sin)
                nc.vector.tensor_tensor(out=trig3, in0=trig3, in1=w2c_b, op=ALU.mult)
                nc.vector.tensor_reduce(out=ne_c[:], in_=trig3, axis=mybir.AxisListType.X, op=ALU.add)

                nc.vector.tensor_tensor(out=ne_s[:], in0=ne_s[:], in1=ne_c[:], op=ALU.add)
                nc.vector.tensor_scalar_add(out=ne_s[:], in0=ne_s[:], scalar1=w0[:])

                # ne placed into fp8 via host mask (16.0 on own jt block):
                # AllGather == AllReduce of zero-padded buffers
                mask_sb = per.tile([128, JT], F32, tag="mask_sb")
                nc.scalar.dma_start(out=mask_sb[:], in_=smap("ne_mask").rearrange("(p j) -> p j", j=JT))
                ne8_placed = per.tile([128, JT], F8, tag="ne8_placed")
                ne_bc3 = ne_s[:].unsqueeze(1).broadcast_to([128, NCORES, JTC])
                nc.vector.tensor_tensor(
                    out=ne8_placed[:].rearrange("p (r b) -> p r b", b=JTC),
                    in0=ne_bc3,
                    in1=mask_sb[:].rearrange("p (r b) -> p r b", b=JTC),
                    op=ALU.mult)
                ne8_lb = per.tile([128, JT], F8, tag="ne8_lb")
                new_e_sb = per.tile([128, JT], F32, tag="new_e_sb")

                # ---------------- LN1 + q/k/v + kTv partial ----------------
                def layer_norm(src, dst, g_bc, b_bc):
                    for nt in range(NT):
                        st = stats.tile([128, 6], F32, tag="ln_st")
                        nc.vector.bn_stats(out=st[:], in_=src[:, nt, :])
                        mv = stats.tile([128, 2], F32, tag="ln_mv")
                        nc.vector.bn_aggr(out=mv[:], in_=st[:])
                        rstd = stats.tile([128, 1], F32, tag="ln_rstd")
                        nc.scalar.activation(out=rstd[:], in_=mv[:, 1:2], func=AF.Sqrt, bias=eps_sb[:])
                        nc.vector.reciprocal(out=rstd[:], in_=rstd[:])
                        nc.vector.tensor_scalar(
                            out=dst[:, nt, :], in0=src[:, nt, :],
                            scalar1=mv[:, 0:1], scalar2=rstd[:],
                            op0=ALU.subtract, op1=ALU.mult,
                        )
                        nc.vector.tensor_tensor(out=dst[:, nt, :], in0=dst[:, nt, :], in1=g_bc[:], op=ALU.mult)
                        nc.vector.tensor_tensor(out=dst[:, nt, :], in0=dst[:, nt, :], in1=b_bc[:], op=ALU.add)

                mh_sb = per.tile([128, NT, C], F32, tag="mh_sb")

                mh16T = per.tile([128, ROWS], BF16, tag="mh16T")
                qT = per.tile([128, ROWS], F32, tag="qT")
                k16_sb = per.tile([128, NT, C], BF16, tag="k16_sb")
                v16_sb = per.tile([128, NT, C], BF16, tag="v16_sb")
                kTv_sb = per.tile([128, C], F32, tag="kTv_sb")

                def emit_qkv_head():
                    layer_norm(h_sb, mh_sb, mhag_bc, mhab_bc)
                    for nt in range(NT):
                        tp = psum_tr.tile([128, 128], F32, tag="tr")
                        nc.tensor.transpose(tp[:], mh_sb[:, nt, :], ident[:])
                        nc.vector.tensor_copy(out=mh16T[:, nt * 128:(nt + 1) * 128], in_=tp[:])
                    for nch in range(ROWS // 512):
                        ps = psum_sm.tile([128, 512], F32, tag="ps_sm")
                        nc.tensor.matmul(ps[:], lhsT=wq_16[:], rhs=mh16T[:, nch * 512:(nch + 1) * 512], start=True, stop=True)
                        nc.scalar.activation(out=qT[:, nch * 512:(nch + 1) * 512], in_=ps[:], func=AF.Identity, bias=bq_pp[:])
                    for nt in range(NT):
                        ps = psum_sm.tile([128, C], F32, tag="ps_sm")
                        nc.tensor.matmul(ps[:], lhsT=mh16T[:, nt * 128:(nt + 1) * 128], rhs=wk_16[:], start=True, stop=True)
                        nc.vector.tensor_add(out=k16_sb[:, nt, :], in0=ps[:], in1=bk_bc[:])
                        ps2 = psum_sm.tile([128, C], F32, tag="ps_sm")
                        nc.tensor.matmul(ps2[:], lhsT=mh16T[:, nt * 128:(nt + 1) * 128], rhs=wv_16[:], start=True, stop=True)
                        nc.vector.tensor_add(out=v16_sb[:, nt, :], in0=ps2[:], in1=bv_bc[:])
                    pskv = psum_sm.tile([128, C], F32, tag="ps_sm")
                    for nt in range(NT):
                        nc.tensor.matmul(pskv[:], lhsT=k16_sb[:, nt, :], rhs=v16_sb[:, nt, :], start=(nt == 0), stop=(nt == NT - 1))
                    nc.vector.tensor_copy(out=kTv_sb[:], in_=pskv[:])

                # ---------------- collectives (triggers on gpsimd only) ----------------
                utxT = per.tile([128, N], F8, tag="utxT")
                ar_ins, ar_outs = [], []
                for c in range(NAR):
                    w = CW + (JT if c == 0 else 0)  # chunk 0 carries ne columns
                    ari = dram.tile([128, w], F8, tag=f"ar_in{c}", name=f"ar_in{c}")
                    aro = dram.tile([128, w], F8, tag=f"ar_out{c}", name=f"ar_out{c}",
                                    addr_space="Shared")
                    ar_ins.append(ari)
                    ar_outs.append(aro)
                ktv_in = dram.tile([128, C], F32, tag="ktv_in")
                ktv_out = dram.tile([128, C], F32, tag="ktv_out", addr_space="Shared")

                def emit_ktv_trigger():
                    nc.sync.dma_start(out=ktv_in[:], in_=kTv_sb[:])
                    nc.gpsimd.collective_compute(
                        "AllReduce", ALU.add, replica_groups=rg,
                        ins=[ktv_in[:].opt()], outs=[ktv_out[:].opt()],
                    )

                def emit_ar_trigger(c):
                    nc.sync.dma_start(out=ar_ins[c][:, 0:CW], in_=utxT[:, c * CW:(c + 1) * CW])
                    if c == 0:
                        nc.sync.dma_start(out=ar_ins[0][:, CW:CW + JT], in_=ne8_placed[:])
                    nc.gpsimd.collective_compute(
                        "AllReduce", ALU.add, replica_groups=rg,
                        ins=[ar_ins[c][:].opt()], outs=[ar_outs[c][:].opt()],
                    )

                # ---------------- pass 1: utx^T = h16^T @ u panels ----------------
                u_r = pk.ap()[OFF_U:OFF_U + ROWS * N].rearrange("(t p j) -> p t j", p=128, j=N)
                if "nopass1" in _ABL:
                    emit_qkv_head()
                else:
                    for jp in range(JP):
                        panel = pan.tile([128, NT, PW], F8, tag="panel")
                        nc.sync.dma_start(out=panel[:], in_=u_r[:, :, jp * PW:(jp + 1) * PW])
                        if jp == min(JPC, JP - 1):
                            emit_qkv_head()
                        if jp >= JPC + 1 and (jp - JPC - 1) % JPC == 0 and (jp - JPC - 1) // JPC < NAR - 1:
                            if "nocoll" not in _ABL:
                                emit_ar_trigger((jp - JPC - 1) // JPC)
                        for jh in range(PW // 512):
                            ps = psum_sm.tile([128, 512], F32, tag="ps_sm")
                            for ntp in range(NT // 2):
                                nc.tensor.matmul(
                                    ps[:], lhsT=h8_sb[:, 2 * ntp:2 * ntp + 2, :],
                                    rhs=panel[:, 2 * ntp:2 * ntp + 2, jh * 512:(jh + 1) * 512],
                                    start=(ntp == 0), stop=(ntp == NT // 2 - 1),
                                    perf_mode=mybir.MatmulPerfMode.DoubleRow,
                                )
                            nc.scalar.activation(
                                out=utxT[:, jp * PW + jh * 512:jp * PW + (jh + 1) * 512],
                                in_=ps[:], func=AF.Identity)
                if "nocoll" not in _ABL and "nopass1" not in _ABL:
                    emit_ar_trigger(NAR - 1)
                    emit_ktv_trigger()

                    # all collective load-backs, in completion order, after the last
                    # panel so they never block the panel stream's FIFO
                    nc.sync.dma_start(out=utxT[:, 0:CW], in_=ar_outs[0][:, 0:CW])
                    nc.sync.dma_start(out=ne8_lb[:], in_=ar_outs[0][:, CW:CW + JT])
                    nc.vector.tensor_copy(out=new_e_sb[:], in_=ne8_lb[:])
                    for c in range(1, NAR):
                        nc.sync.dma_start(out=utxT[:, c * CW:(c + 1) * CW], in_=ar_outs[c][:])
                    nc.sync.dma_start(out=kTv_sb[:], in_=ktv_out[:])
                elif "nopass2" not in _ABL:
                    nc.vector.memset(new_e_sb[:], 0.0)
                    if "nopass1" in _ABL:
                        nc.vector.memset(utxT[:], 0.0)

                # ---------------- pass 2: h_fur^T = sum_jt g16[jt]^T @ uT16[jt] ----------------
                uT_r = pk.ap()[OFF_UT:OFF_UT + N * ROWS].rearrange("(jt p r) -> p jt r", p=128, r=ROWS)
                hfur_ps = psum_acc.tile([128, ROWS], F32, tag="hfur")
                if "nopass2" not in _ABL:
                    for jtg in range(JT // UTG):
                        ut = u16t_pool.tile([128, UTG, ROWS], F8, tag="ut")
                        nc.scalar.dma_start(out=ut[:], in_=uT_r[:, jtg * UTG:(jtg + 1) * UTG, :])
                        for jpr in range(UTG // 2):
                            g8p = g16_pool.tile([128, 2, 128], F8, tag="g8p")
                            for k in range(2):
                                jt = jtg * UTG + jpr * 2 + k
                                t16 = t16_pool.tile([128, 128], BF16, tag="t16")
                                nc.vector.tensor_copy(out=t16[:], in_=utxT[:, jt * 128:(jt + 1) * 128])
                                tp = psum_tr.tile([128, 128], BF16, tag="tr", name="tp16")
                                nc.tensor.transpose(tp[:], t16[:], ident16[:])
                                nc.vector.tensor_scalar_mul(out=g8p[:, k, :], in0=tp[:], scalar1=new_e_sb[:, jt:jt + 1])
                            pair = jtg * (UTG // 2) + jpr
                            for hf in range(ROWS // 512):
                                nc.tensor.matmul(
                                    hfur_ps[:, hf * 512:(hf + 1) * 512], lhsT=g8p[:],
                                    rhs=ut[:, jpr * 2:jpr * 2 + 2, hf * 512:(hf + 1) * 512],
                                    start=(pair == 0), stop=(pair == JT // 2 - 1),
                                    skip_group_check=True,
                                    perf_mode=mybir.MatmulPerfMode.DoubleRow,
                                )
                else:
                    nc.vector.memset(hfur_ps[:], 0.0)

                # ---------------- att^T (fp32r), s^T, h1 ----------------
                hfurT = per.tile([128, ROWS], F32, tag="hfurT")
                nc.vector.tensor_scalar_mul(out=hfurT[:], in0=hfur_ps[:], scalar1=1.0 / 4096.0)

                attT = per.tile([128, ROWS], F32, tag="attT")
                for nch in range(ROWS // 512):
                    ps = psum_sm.tile([128, 512], F32, tag="ps_sm")
                    nc.tensor.matmul(ps[:], lhsT=kTv_sb[:],
                                     rhs=qT[:, nch * 512:(nch + 1) * 512],
                                     start=True, stop=True)
                    nc.vector.tensor_copy(out=attT[:, nch * 512:(nch + 1) * 512], in_=ps[:])

                sT = per.tile([128, ROWS], F32, tag="sT")
                for nch in range(ROWS // 512):
                    ps = psum_sm.tile([128, 512], F32, tag="ps_sm")
                    nc.tensor.matmul(ps[:], lhsT=wo_sb[:],
                                     rhs=attT[:, nch * 512:(nch + 1) * 512],
                                     start=True, stop=True)
                    nc.vector.scalar_tensor_tensor(
                        out=sT[:, nch * 512:(nch + 1) * 512], in0=ps[:], scalar=bo_pp[:],
                        in1=hfurT[:, nch * 512:(nch + 1) * 512],
                        op0=ALU.add, op1=ALU.add,
                    )

                h1_sb = per.tile([128, NT, C], F32, tag="h1_sb")
                for nt in range(NT):
                    tp = psum_tr.tile([128, 128], F32, tag="tr")
                    nc.tensor.transpose(tp[:], sT[:, nt * 128:(nt + 1) * 128], ident[:])
                    nc.vector.tensor_add(out=h1_sb[:, nt, :], in0=tp[:], in1=h_sb[:, nt, :])

                # ---------------- FFN ----------------
                mh2_sb = per.tile([128, NT, C], F32, tag="mh2_sb")
                layer_norm(h1_sb, mh2_sb, ffng_bc, ffnb_bc)
                mh2T = per.tile([128, ROWS], BF16, tag="mh2T")
                for nt in range(NT):
                    tp = psum_tr.tile([128, 128], F32, tag="tr")
                    nc.tensor.transpose(tp[:], mh2_sb[:, nt, :], ident[:])
                    nc.vector.tensor_copy(out=mh2T[:, nt * 128:(nt + 1) * 128], in_=tp[:])

                gzT = per.tile([128, ROWS], BF16, tag="gzT")
                for nch in range(ROWS // 512):
                    ps = psum_sm.tile([128, 512], F32, tag="ps_sm")
                    nc.tensor.matmul(ps[:], lhsT=f1w_16[:],
                                     rhs=mh2T[:, nch * 512:(nch + 1) * 512],
                                     start=True, stop=True)
                    nc.scalar.activation(out=gzT[:, nch * 512:(nch + 1) * 512], in_=ps[:], func=AF.Gelu, bias=f1b_pp[:])

                f2T = per.tile([128, ROWS], F32, tag="f2T")
                for nch in range(ROWS // 512):
                    ps = psum_sm.tile([128, 512], F32, tag="ps_sm")
                    nc.tensor.matmul(ps[:], lhsT=f2w_16[:],
                                     rhs=gzT[:, nch * 512:(nch + 1) * 512],
                                     start=True, stop=True)
                    nc.scalar.activation(out=f2T[:, nch * 512:(nch + 1) * 512], in_=ps[:], func=AF.Identity, bias=f2b_pp[:])

                hout_sb = per.tile([128, NT, C], F32, tag="hout_sb")
                for nt in range(NT):
                    tp = psum_tr.tile([128, 128], F32, tag="tr")
                    nc.tensor.transpose(tp[:], f2T[:, nt * 128:(nt + 1) * 128], ident[:])
                    nc.vector.tensor_add(out=hout_sb[:, nt, :], in0=tp[:], in1=h1_sb[:, nt, :])

                nc.sync.dma_start(out=y.ap().rearrange("(t p) c -> p t c", p=128), in_=hout_sb[:])

            for _rep in range(_REPL):
                _body_once()

    nc.compile()
    return nc


_NC = None


def _get_nc():
    global _NC
    if _NC is None:
        _NC = _build(bacc.Bacc("TRN2", target_bir_lowering=False, debug=False, num_devices=NCORES))
    return _NC


def make_in_maps(inputs):
    import ml_dtypes
    BF = ml_dtypes.bfloat16
    F8E4 = ml_dtypes.float8_e4m3

    e = np.ascontiguousarray(np.asarray(inputs["e"], dtype=np.float32))
    u = np.asarray(inputs["u"], dtype=np.float32)
    x = np.asarray(inputs["x"], dtype=np.float32)
    e_resh = np.ascontiguousarray(e.reshape(JT, 128).T)  # [p, jt] = e[jt*128+p]

    sm_common = np.zeros(SM_TOTAL, np.float32)
    for name, _ in WEIGHT_NAMES:
        o, n = SM_OFF[name]
        sm_common[o:o + n] = np.asarray(inputs[name], dtype=np.float32).ravel()

    in_maps = []
    for m in range(NCORES):
        us = u[m * ROWS:(m + 1) * ROWS]
        xs = x[m * ROWS:(m + 1) * ROWS]
        us64 = us * np.float32(16.0)

        sm = sm_common.copy()
        o, n = SM_OFF["e_js"]
        sm[o:o + n] = np.ascontiguousarray(e_resh[:, m * JTC:(m + 1) * JTC]).ravel()
        mask = np.zeros((128, JT), np.float32)
        mask[:, m * JTC:(m + 1) * JTC] = 16.0
        o, n = SM_OFF["ne_mask"]
        sm[o:o + n] = mask.ravel()

        buf = np.empty(PK_BYTES, np.uint8)
        buf[OFF_U:OFF_U + ROWS * N] = us64.astype(F8E4).view(np.uint8).ravel()
        buf[OFF_UT:OFF_UT + N * ROWS] = np.ascontiguousarray(us64.T).astype(F8E4).view(np.uint8).ravel()
        buf[OFF_X:OFF_X + NFEAT * ROWS * 2] = np.ascontiguousarray(xs.T).astype(BF).view(np.uint8).ravel()
        buf[OFF_SM:] = sm.view(np.uint8)
        in_maps.append({"pk": buf.view(F8E4)})
    return in_maps


def kernel(**inputs):
    nc = _get_nc()
    in_maps = make_in_maps(inputs)

    trace = bool(int(_os.environ.get("KERNEL_TRACE", "0")))
    res = run_bass_kernel_spmd(nc, in_maps, core_ids=list(range(NCORES)), trace=trace)
    if trace and res.exec_time_ns is not None:
        print(f"HW exec time: {res.exec_time_ns} ns")
        if res.instructions_and_trace is not None:
            print("trace:", res.instructions_and_trace[1])
    out = np.concatenate([r["y"] for r in res.results], axis=0)
    return out.astype(np.float32)

